# revision 23
# baseline (speedup 1.0000x reference)
"""4-branch bidirectional GRU (nn_RNN_2817498546846) on 8 TRN2 NeuronCores.

Sharding: core i handles cell k=i//2 (air0,bed0,air1,bed1) and batch half
j=i%2 (256 rows). Weights per-core = one cell only; no collectives.
Cells 2,3 consume the time-reversed sequence -> host reverses their data,
so the device program is identical on all cores (pure SPMD).

Mixed-precision edition: the x-side gate matmuls (r/z/n input projections
against Wih) run as fp8e4m3 DoubleRow matmuls (2 contraction rows/cycle,
x scaled by 16, Wih by 2048); the h-side matmuls (Whh) stay bf16 for
recurrence accuracy, with Whh pre-scaled by 2^15 on the host so both
sides accumulate in the same PSUM domain (descaled at the activations).
r/z biases enter PSUM via one K=2 bias-row matmul per bank so each r/z
bank is consumed by ONE full-bank [128,512] sigmoid; full-bank PSUM reads
also make bank recycling WAR-sound (start=True wipes its whole bank).
Step emission order is arranged so the x-phase of step t+1 reuses banks
in the order the chain of step t releases them (rz first, ghn/proj mid,
gin last), keeping PE warm through the serial chain.
"""

import sys
import numpy as np

sys.path.insert(0, "/opt/trn_rl_repo")

import ml_dtypes

B, F, T, H, K = 512, 64, 64, 512, 4
BL = 256          # batch per core
NBLK, SPB = 8, 8  # 8 blocks x 8 steps
BF16 = ml_dtypes.bfloat16
F8 = ml_dtypes.float8_e4m3   # TRN fp8e4 (IEEE-ish, max 240)

SX = 16.0      # x quantization scale
SW = 2048.0    # Wih quantization scale
PS = SX * SW   # psum scale (2^15)
DS = 1.0 / PS

_CACHE = {}
TRACE = False   # test harness sets True to capture NTFF profile
LAST = {}       # stashes the BassKernelResults of the most recent run


def _legalize(nc, mybir):
    """Walrus codegen allows at most ONE embedded sem wait per instruction
    (libwalrus setupSyncWait asserts count==1 for every ISA struct). Engines
    execute their streams in order, so extra waits move onto same-engine
    NoOps inserted immediately before the offending instruction."""
    n_split = 0
    for f in nc.m.functions:
        for b in f.blocks:
            insts = b.instructions
            out = []
            for ins in insts:
                si = getattr(ins, "sync_info", None)
                waits = list(si.on_wait) if si is not None and si.on_wait else []
                if len(waits) > 1:
                    for k, w in enumerate(waits[:-1]):
                        nop = mybir.InstNoOp(
                            name=f"{ins.name}-lw{k}",
                            engine=ins.engine,
                            bass_nofuse=True,
                            sync_info=mybir.SyncInfo(on_wait=[w], on_update=[]),
                        )
                        out.append(nop)
                        n_split += 1
                    ups = list(si.on_update) if si.on_update else []
                    ins.sync_info = mybir.SyncInfo(on_wait=[waits[-1]], on_update=ups)
                out.append(ins)
            insts[:] = out
    return n_split


def _build():
    import concourse.bass as bass
    import concourse.tile as tile
    from concourse import mybir

    dt = mybir.dt
    AF = mybir.ActivationFunctionType
    OP = mybir.AluOpType
    DR = mybir.MatmulPerfMode.DoubleRow

    nc = bass.Bass("TRN2", target_bir_lowering=False, debug=False, num_devices=8)

    FB = F + 1  # input feature rows + ones row (carries fc_in bias)
    xdat_d = nc.declare_dram_parameter("xdat", [FB, T * BL], dt.bfloat16,
                                       isOutput=False)
    fcin_d = nc.declare_dram_parameter("fcin", [FB, H], dt.bfloat16, isOutput=False)
    wi_d = [nc.declare_dram_parameter(f"wi{p}", [128, 2 * 3 * H], dt.float8e4,
                                      isOutput=False) for p in range(2)]
    wh8_d = [nc.declare_dram_parameter(f"wh8{p}", [128, 2 * 2 * H], dt.float8e4,
                                       isOutput=False) for p in range(2)]
    whh_d = nc.declare_dram_parameter("whh", [128, 4 * H], dt.bfloat16,
                                      isOutput=False)
    brow_d = nc.declare_dram_parameter("brow", [2, 512], dt.bfloat16, isOutput=False)
    sel_d = nc.declare_dram_parameter("sel", [2, 512], dt.bfloat16, isOutput=False)
    bni_d = nc.declare_dram_parameter("bni", [128, 4], dt.float32, isOutput=False)
    brown_d = nc.declare_dram_parameter("brown", [2, 256], dt.bfloat16,
                                        isOutput=False)
    h0_d = nc.declare_dram_parameter("h0", [H, BL], dt.float32, isOutput=False)
    wout_d = nc.declare_dram_parameter("wout", [128, 32], dt.bfloat16,
                                       isOutput=False)
    yout_d = nc.declare_dram_parameter("yout", [T, BL], dt.float32, isOutput=True)

    with tile.TileContext(nc) as tc:
        with (
            tc.tile_pool(name="wpool", bufs=1) as wpool,
            tc.tile_pool(name="xpool", bufs=2) as xpool,
            tc.tile_pool(name="hpool", bufs=1) as hpool,
            tc.tile_pool(name="tpool", bufs=4) as tpool,
            tc.tile_pool(name="ppool", bufs=2, space=bass.MemorySpace.PSUM) as ppool,
        ):
            # ---- persistent constants ----
            wi = [wpool.tile([128, 2, 3 * H], dt.float8e4, name=f"wi{p}",
                             tag=f"wi{p}") for p in range(2)]
            wh8 = [wpool.tile([128, 2, 2 * H], dt.float8e4, name=f"wh8{p}",
                              tag=f"wh8{p}") for p in range(2)]
            # whh: n-gate cols only [k-chunk, kc*512 + col], values * PS
            whh = wpool.tile([128, 4 * H], dt.bfloat16, name="whh", tag="whh")
            fcin = wpool.tile([FB, H], dt.bfloat16, name="fcin", tag="fcin")
            stg = wpool.tile([FB, T * BL], dt.bfloat16, name="stg", tag="stg")
            brow = wpool.tile([2, 512], dt.bfloat16, name="brow", tag="brow")
            sel = wpool.tile([2, 512], dt.bfloat16, name="sel", tag="sel")
            bni = wpool.tile([128, 4], dt.float32, name="bni", tag="bni")
            brown = wpool.tile([2, 256], dt.bfloat16, name="brown", tag="brown")
            wout = wpool.tile([128, 32], dt.bfloat16, name="wout", tag="wout")
            fcint = wpool.tile([FB, H], dt.bfloat16, name="fcint", tag="fcint")
            h_all = hpool.tile([128, 4 * BL], dt.float32, name="hall", tag="hall")
            hb = hpool.tile([128, 4, BL], dt.bfloat16, name="hb", tag="hb")
            hb8 = hpool.tile([128, 4, BL], dt.float8e4, name="hb8", tag="hb8")

            CW = SPB * BL  # columns per block

            # early DMAs: block-0 inputs + projection weights first so PE can
            # start the block-0 projection while the big weight DMAs land.
            nc.sync.dma_start(stg[:FB, 0:CW], xdat_d[:, 0:CW])
            nc.sync.dma_start(fcint[:FB, :], fcin_d[:])
            # DVE funnel: PE Matmult supports only ONE embedded sem wait, so
            # route DMA-landed matmul operands through DVE; matmul deps then
            # collapse onto the single DVE semaphore.
            nc.vector.tensor_copy(fcin[:FB, :], fcint[:FB, :])
            nc.sync.dma_start(stg[:FB, CW:2 * CW], xdat_d[:, CW:2 * CW])
            for c in range(4):
                nc.sync.dma_start(h_all[:, c * BL:(c + 1) * BL],
                                  h0_d[c * 128:(c + 1) * 128, :])
            nc.vector.tensor_copy(hb[:], h_all[:])
            nc.vector.tensor_scalar_mul(hb8[:], h_all[:], SX)
            nc.sync.dma_start(brow[:], brow_d[:])
            nc.sync.dma_start(sel[:], sel_d[:])
            nc.sync.dma_start(bni[:], bni_d[:])
            nc.sync.dma_start(brown[:], brown_d[:])
            nc.sync.dma_start(wout[:], wout_d[:])
            nc.sync.dma_start(whh[:], whh_d[:])
            for p in range(2):
                nc.sync.dma_start(wi[p][:], wi_d[p][:])
                nc.sync.dma_start(wh8[p][:], wh8_d[p][:])

            def whn(kc, c4):
                # Whh.T n-gate block: contraction chunk kc, out-chunk c4
                return whh[:, kc * H + c4 * 128: kc * H + (c4 + 1) * 128]

            def proj_col(off, xb_t, s):
                # two oc per PSUM bank, ONE accumulation group per bank,
                # then one full-bank relu evicting both chunks as scaled fp8
                for half in range(2):
                    pj = ppool.tile([128, 2 * BL], dt.float32, name="gpj",
                                    tag="gp", bufs=2)
                    for oc2 in range(2):
                        oc = half * 2 + oc2
                        nc.tensor.matmul(pj[:, oc2 * BL:(oc2 + 1) * BL],
                                         fcin[:FB, oc * 128:(oc + 1) * 128],
                                         stg[:FB, off + s * BL:off + (s + 1) * BL],
                                         start=(oc2 == 0), stop=(oc2 == 1),
                                         skip_group_check=True)
                    nc.scalar.activation(
                        xb_t[:, 2 * half:2 * half + 2, s * BL:(s + 1) * BL],
                        pj[:], AF.Relu, scale=SX)

            # block 0 projects its own inputs up front (PE filler during the
            # weight DMAs); later blocks are projected inside the prior block
            xb_cur = xpool.tile([128, 4, CW], dt.float8e4, name="xb8",
                                tag="xb8", bufs=2)
            for s in range(SPB):
                proj_col(0, xb_cur, s)

            pend = None
            for blk in range(NBLK):
                if blk + 1 < NBLK:
                    xb_next = xpool.tile([128, 4, CW], dt.float8e4, name="xb8",
                                         tag="xb8", bufs=2)
                if blk + 2 < NBLK:
                    nc.sync.dma_start(stg[:FB, (blk + 2) * CW:(blk + 3) * CW],
                                      xdat_d[:, (blk + 2) * CW:(blk + 3) * CW])
                ysb = tpool.tile([1, SPB * BL], dt.float32, name="ysb",
                                 tag="ysb", bufs=2)

                for s in range(SPB):
                    def xsl(p):
                        return xb_cur[:, 2 * p:2 * p + 2, s * BL:(s + 1) * BL]

                    # ---- x-phase: banks in the order chain(t-1) frees them
                    # r/z x-side + K=2 bias row opens each bank's group;
                    # rz banks were freed by the earliest chain events
                    # (the sigmoids), so PE restarts immediately.
                    rz_ps = []
                    for bk in range(4):  # banks: r01, r23, z01, z23
                        gp = ppool.tile([128, 2 * BL], dt.float32, name="grz",
                                        tag="grz", bufs=4)
                        nc.tensor.matmul(gp[:], brow[:, bk * 128:(bk + 1) * 128],
                                         sel[:], start=True, stop=False,
                                         skip_group_check=True)
                        for c2 in range(2):
                            m = bk * 2 + c2
                            ms = slice(m * 128, (m + 1) * 128)
                            for p in range(2):
                                nc.tensor.matmul(
                                    gp[:, c2 * BL:(c2 + 1) * BL],
                                    wi[p][:, :, ms], xsl(p),
                                    start=False, stop=False,
                                    perf_mode=DR, skip_group_check=True)
                        rz_ps.append(gp)

                    # next block's projection (gp banks, freed by rhn)
                    if blk + 1 < NBLK:
                        proj_col((blk + 1) * CW, xb_next, s)

                    # deferred output head for the previous step (gp bank)
                    if pend is not None:
                        pysb, pblk, ps_ = pend
                        yp = ppool.tile([128, 2 * BL], dt.float32, name="yp",
                                        tag="gp", bufs=2)
                        for c in range(4):
                            w0 = ps_ * 4 + c
                            nc.tensor.matmul(yp[0:1, 0:BL],
                                             wout[:, w0:w0 + 1],
                                             hb[:, c, :],
                                             start=(c == 0), stop=(c == 3),
                                             skip_group_check=True)
                        nc.scalar.activation(pysb[0:1, ps_ * BL:(ps_ + 1) * BL],
                                             yp[0:1, 0:BL], AF.Copy)
                        if ps_ == SPB - 1:
                            nc.sync.dma_start(
                                yout_d[pblk * SPB:(pblk + 1) * SPB, :],
                                pysb[0:1, :])
                        pend = None

                    # n-gate input side (gin): own banks, freed last (by sa)
                    gin_ps = []
                    for pr in range(2):
                        gp = ppool.tile([128, 2 * BL], dt.float32, name="gin",
                                        tag="gin", bufs=2)
                        for c2 in range(2):
                            c4 = pr * 2 + c2
                            ms = slice((8 + c4) * 128, (9 + c4) * 128)
                            for p in range(2):
                                nc.tensor.matmul(
                                    gp[:, c2 * BL:(c2 + 1) * BL],
                                    wi[p][:, :, ms], xsl(p),
                                    start=(c2 == 0 and p == 0),
                                    stop=(c2 == 1 and p == 1),
                                    perf_mode=DR, skip_group_check=True)
                        gin_ps.append(gp)

                    # ---- h-phase (bf16): r banks + ghn first so the chain
                    # starts early; z banks feed the late h-update
                    r_all = tpool.tile([128, 4 * BL], dt.float32, name="rall",
                                       tag="rall", bufs=2)
                    z_all = tpool.tile([128, 4 * BL], dt.float32, name="zall",
                                       tag="zall", bufs=2)
                    zp_all = tpool.tile([128, 4 * BL], dt.float32, name="zpall",
                                        tag="zpall", bufs=2)
                    ghn_ps = []

                    def rz_h(bk):
                        gp = rz_ps[bk]
                        for p in range(2):
                            for c2 in range(2):
                                m = bk * 2 + c2
                                ms = slice(m * 128, (m + 1) * 128)
                                nc.tensor.matmul(
                                    gp[:, c2 * BL:(c2 + 1) * BL],
                                    wh8[p][:, :, ms],
                                    hb8[:, 2 * p:2 * p + 2, :],
                                    start=False,
                                    stop=(c2 == 1 and p == 1),
                                    perf_mode=DR, skip_group_check=True)
                        o = (bk % 2) * 2 * BL
                        if bk < 2:
                            nc.scalar.activation(r_all[:, o:o + 2 * BL], gp[:],
                                                 AF.Sigmoid, scale=DS)
                        else:
                            nc.scalar.activation(z_all[:, o:o + 2 * BL], gp[:],
                                                 AF.Sigmoid, scale=DS)
                            nc.scalar.activation(zp_all[:, o:o + 2 * BL], gp[:],
                                                 AF.Sigmoid, scale=-DS)

                    def ghn_h(pr):
                        gp = ppool.tile([128, 2 * BL], dt.float32, name="ghn",
                                        tag="gp", bufs=2)
                        nc.tensor.matmul(gp[:], brown[:, pr * 128:(pr + 1) * 128],
                                         sel[:], start=True, stop=False,
                                         skip_group_check=True)
                        for c2 in range(2):
                            c4 = pr * 2 + c2
                            for kc in range(4):
                                nc.tensor.matmul(
                                    gp[:, c2 * BL:(c2 + 1) * BL],
                                    whn(kc, c4), hb[:, kc, :],
                                    start=False,
                                    stop=(c2 == 1 and kc == 3),
                                    skip_group_check=True)
                        ghn_ps.append(gp)

                    rz_h(0)      # r01 -> sigmoid r01
                    ghn_h(0)     # ghn01
                    rz_h(1)      # r23 -> sigmoid r23
                    ghn_h(1)     # ghn23
                    rz_h(2)      # z01
                    rz_h(3)      # z23

                    t1 = tpool.tile([128, 4 * BL], dt.float32, name="t1",
                                    tag="t1", bufs=2)
                    for c4 in range(4):
                        cs = slice(c4 * BL, (c4 + 1) * BL)
                        nc.gpsimd.tensor_mul(t1[:, cs], z_all[:, cs],
                                             hb[:, c4:c4 + 1, :])

                    # ---- chain: rhn = ghn*r (full-bank, bnh already in
                    # psum via the bias row), sa = gin + rhn (full-bank),
                    # tanh per chunk (bias bni), then the ladder-free update
                    # h' = z*h + (1-z)*n with 1-z = sigmoid(-x) from the same
                    # bank; hb recast per pair so pair0 overlaps pair1
                    sa_all = tpool.tile([128, 4 * BL], dt.float32, name="saall",
                                        tag="saall", bufs=2)
                    nsb_all = tpool.tile([128, 4 * BL], dt.float32, name="nsball",
                                         tag="nsball", bufs=2)
                    t2 = tpool.tile([128, 4 * BL], dt.float32, name="t2",
                                    tag="t2", bufs=2)
                    # all four full-bank PSUM reads lead the DVE stream so
                    # pair1's chain starts as soon as its banks stop (not
                    # queued behind pair0's elementwise tail)
                    for pr in range(2):
                        o = pr * 2 * BL
                        rhn = tpool.tile([128, 2 * BL], dt.float32, name="rhn",
                                         tag="rhn", bufs=2)
                        nc.vector.tensor_mul(rhn[:], ghn_ps[pr][:],
                                             r_all[:, o:o + 2 * BL])
                        nc.vector.tensor_add(sa_all[:, o:o + 2 * BL],
                                             gin_ps[pr][:], rhn[:])
                    for pr in range(2):
                        for c2 in range(2):
                            c4 = pr * 2 + c2
                            cs = slice(c4 * BL, (c4 + 1) * BL)
                            nc.scalar.activation(nsb_all[:, cs], sa_all[:, cs],
                                                 AF.Tanh, bias=bni[:, c4:c4 + 1],
                                                 scale=DS)
                    for pr in range(2):
                        for c2 in range(2):
                            c4 = pr * 2 + c2
                            cs = slice(c4 * BL, (c4 + 1) * BL)
                            nc.vector.tensor_mul(t2[:, cs], zp_all[:, cs],
                                                 nsb_all[:, cs])
                            nc.vector.tensor_add(hb[:, c4:c4 + 1, :],
                                                 t1[:, cs], t2[:, cs])
                        # hb8 gates the next step's rz-h matmuls
                        nc.vector.tensor_scalar_mul(
                            hb8[:, 2 * pr:2 * pr + 2, :],
                            hb[:, 2 * pr:2 * pr + 2, :], SX)
                    pend = (ysb, blk, s)

                if blk + 1 < NBLK:
                    xb_cur = xb_next

            # drain the final step's head
            pysb, pblk, ps_ = pend
            yp = ppool.tile([128, 2 * BL], dt.float32, name="yp", tag="gp",
                            bufs=2)
            for c in range(4):
                w0 = ps_ * 4 + c
                nc.tensor.matmul(yp[0:1, 0:BL], wout[:, w0:w0 + 1],
                                 hb[:, c, :], start=(c == 0), stop=(c == 3),
                                 skip_group_check=True)
            nc.scalar.activation(pysb[0:1, ps_ * BL:(ps_ + 1) * BL],
                                 yp[0:1, 0:BL], AF.Copy)
            nc.sync.dma_start(yout_d[pblk * SPB:(pblk + 1) * SPB, :],
                              pysb[0:1, :])

    _legalize(nc, mybir)
    return nc


def _get_nc():
    if "nc" not in _CACHE:
        _CACHE["nc"] = _build()
    return _CACHE["nc"]


def _wsp(w):
    chunks = w.reshape(4, 128)
    out = np.zeros((128, 32), np.float32)
    for s in range(SPB):
        for c in range(4):
            out[:, s * 4 + c] = chunks[c]
    return out.astype(BF16)


def _pack_dr(wT):
    """[512, 1536] contraction-major weight -> two DoubleRow pair tensors
    [128, 2*1536] fp8: pair p rows (2p,2p+1); [k, i*1536+m] = wT[p*256+i*128+k, m]."""
    w = (wT * SW).astype(np.float32)
    out = []
    for p in range(2):
        t = np.empty((128, 2 * 3 * H), np.float32)
        for i in range(2):
            t[:, i * 3 * H:(i + 1) * 3 * H] = w[(2 * p + i) * 128:
                                                (2 * p + i + 1) * 128, :]
        out.append(np.ascontiguousarray(t).astype(F8))
    return out


def kernel(data, init, fc_in_W, fc_in_b, Wih, Whh, bih, bhh, fc_out_W, fc_out_b):
    from concourse.bass_utils import run_bass_kernel_spmd

    data = np.asarray(data, np.float32)
    init = np.asarray(init, np.float32)
    fc_in_W = np.asarray(fc_in_W, np.float32)
    fc_in_b = np.asarray(fc_in_b, np.float32)
    Wih = np.asarray(Wih, np.float32)
    Whh = np.asarray(Whh, np.float32)
    bih = np.asarray(bih, np.float32)
    bhh = np.asarray(bhh, np.float32)
    fc_out_W = np.asarray(fc_out_W, np.float32)
    fc_out_b = np.asarray(fc_out_b, np.float32)

    nc = _get_nc()

    sel = np.zeros((2, 512), np.float32)
    sel[0, :256] = 1.0
    sel[1, 256:] = 1.0
    sel = sel.astype(BF16)

    in_maps = []
    for i in range(8):
        k, j = i // 2, i % 2
        d = data[j * BL:(j + 1) * BL]            # [256, 64, 64] (b,f,t)
        if k >= 2:
            d = d[:, :, ::-1]                    # reversed-time branches
        xdat = np.ascontiguousarray(d.transpose(1, 2, 0)).reshape(F, T * BL)
        xdat = np.concatenate([xdat, np.ones((1, T * BL), np.float32)], axis=0)
        fcin = np.concatenate([fc_in_W[k].T, fc_in_b[k][None, :]], axis=0)
        brz = (bih[k][:2 * H] + bhh[k][:2 * H]) * PS     # [1024]
        brow = brz.reshape(8, 128)               # chunk-major
        brow2 = np.empty((2, 512), np.float32)
        for bk in range(4):
            for i2 in range(2):
                brow2[i2, bk * 128:(bk + 1) * 128] = brow[2 * bk + i2]
        bnhr = (bhh[k][2 * H:] * PS).reshape(4, 128)
        brown_n = np.empty((2, 256), np.float32)
        for pr in range(2):
            for i2 in range(2):
                brown_n[i2, pr * 128:(pr + 1) * 128] = bnhr[2 * pr + i2]
        brown_n = np.ascontiguousarray(brown_n).astype(BF16)
        wip = _pack_dr(Wih[k].T)
        whT_rz = (Whh[k].T[:, :2 * H] * SW).astype(np.float32)  # [512, 1024]
        wh8p = []
        for p in range(2):
            t = np.empty((128, 2 * 2 * H), np.float32)
            for i2 in range(2):
                t[:, i2 * 2 * H:(i2 + 1) * 2 * H] = whT_rz[(2 * p + i2) * 128:
                                                           (2 * p + i2 + 1) * 128]
            wh8p.append(np.ascontiguousarray(t).astype(F8))
        # whh: n cols only, [128, kc*512 + m] = Whh.T[kc*128+kk, 2H+m] * PS
        whT = (Whh[k].T[:, 2 * H:] * PS).astype(np.float32)  # [512, 512]
        whh = np.empty((128, 4 * H), np.float32)
        for kc in range(4):
            whh[:, kc * H:(kc + 1) * H] = whT[kc * 128:(kc + 1) * 128]
        in_maps.append({
            "xdat": np.ascontiguousarray(xdat).astype(BF16),
            "fcin": np.ascontiguousarray(fcin).astype(BF16),  # [65, 512]
            "wi0": wip[0], "wi1": wip[1],
            "wh80": wh8p[0], "wh81": wh8p[1],
            "whh": np.ascontiguousarray(whh).astype(BF16),
            "brow": np.ascontiguousarray(brow2).astype(BF16),
            "sel": sel,
            "bni": np.ascontiguousarray(bih[k][2 * H:].reshape(4, 128).T),
            "brown": brown_n,
            "h0": np.ascontiguousarray(init[j * BL:(j + 1) * BL].T),
            "wout": _wsp(fc_out_W[k % 2]),
        })

    kw = {"trace": True} if TRACE else {}
    res = run_bass_kernel_spmd(nc, in_maps, list(range(8)), **kw)
    LAST["res"] = res
    y = [np.asarray(res.results[i]["yout"], np.float32) for i in range(8)]

    air_out = np.empty((B, T), np.float32)
    bed_out = np.empty((B, T), np.float32)
    for j in range(2):
        sl = slice(j * BL, (j + 1) * BL)
        air_out[sl] = (y[0 + j] + y[4 + j][::-1]).T + fc_out_b[0]
        bed_out[sl] = (y[2 + j] + y[6 + j][::-1]).T + fc_out_b[1]
    return air_out, bed_out


# revision 24
# speedup vs baseline: 1.0487x; 1.0487x over previous
"""4-branch bidirectional GRU (nn_RNN_2817498546846) on 8 TRN2 NeuronCores.

Sharding: core i handles cell k=i//2 (air0,bed0,air1,bed1) and batch half
j=i%2 (256 rows). Weights per-core = one cell only; no collectives.
Cells 2,3 consume the time-reversed sequence -> host reverses their data,
so the device program is identical on all cores (pure SPMD).

Mixed-precision edition: the x-side gate matmuls (r/z/n input projections
against Wih) run as fp8e4m3 DoubleRow matmuls (2 contraction rows/cycle,
x scaled by 16, Wih by 2048); the h-side matmuls (Whh) stay bf16 for
recurrence accuracy, with Whh pre-scaled by 2^15 on the host so both
sides accumulate in the same PSUM domain (descaled at the activations).
r/z biases enter PSUM via one K=2 bias-row matmul per bank so each r/z
bank is consumed by ONE full-bank [128,512] sigmoid; full-bank PSUM reads
also make bank recycling WAR-sound (start=True wipes its whole bank).
Step emission order is arranged so the x-phase of step t+1 reuses banks
in the order the chain of step t releases them (rz first, ghn/proj mid,
gin last), keeping PE warm through the serial chain.
"""

import sys
import numpy as np

sys.path.insert(0, "/opt/trn_rl_repo")

import ml_dtypes

B, F, T, H, K = 512, 64, 64, 512, 4
BL = 256          # batch per core
NBLK, SPB = 8, 8  # 8 blocks x 8 steps
BF16 = ml_dtypes.bfloat16
F8 = ml_dtypes.float8_e4m3   # TRN fp8e4 (IEEE-ish, max 240)

SX = 16.0      # x quantization scale
SW = 2048.0    # Wih quantization scale
PS = SX * SW   # psum scale (2^15)
DS = 1.0 / PS

_CACHE = {}
TRACE = False   # test harness sets True to capture NTFF profile
LAST = {}       # stashes the BassKernelResults of the most recent run


def _legalize(nc, mybir):
    """Walrus codegen allows at most ONE embedded sem wait per instruction
    (libwalrus setupSyncWait asserts count==1 for every ISA struct). Engines
    execute their streams in order, so extra waits move onto same-engine
    NoOps inserted immediately before the offending instruction."""
    n_split = 0
    for f in nc.m.functions:
        for b in f.blocks:
            insts = b.instructions
            out = []
            for ins in insts:
                si = getattr(ins, "sync_info", None)
                waits = list(si.on_wait) if si is not None and si.on_wait else []
                if len(waits) > 1:
                    for k, w in enumerate(waits[:-1]):
                        nop = mybir.InstNoOp(
                            name=f"{ins.name}-lw{k}",
                            engine=ins.engine,
                            bass_nofuse=True,
                            sync_info=mybir.SyncInfo(on_wait=[w], on_update=[]),
                        )
                        out.append(nop)
                        n_split += 1
                    ups = list(si.on_update) if si.on_update else []
                    ins.sync_info = mybir.SyncInfo(on_wait=[waits[-1]], on_update=ups)
                out.append(ins)
            insts[:] = out
    return n_split


def _build():
    import concourse.bass as bass
    import concourse.tile as tile
    from concourse import mybir

    dt = mybir.dt
    AF = mybir.ActivationFunctionType
    OP = mybir.AluOpType
    DR = mybir.MatmulPerfMode.DoubleRow

    nc = bass.Bass("TRN2", target_bir_lowering=False, debug=False, num_devices=8)

    FB = F + 1  # input feature rows + ones row (carries fc_in bias)
    xdat_d = nc.declare_dram_parameter("xdat", [FB, T * BL], dt.bfloat16,
                                       isOutput=False)
    fcin_d = nc.declare_dram_parameter("fcin", [FB, H], dt.bfloat16, isOutput=False)
    wi_d = [nc.declare_dram_parameter(f"wi{p}", [128, 2 * 3 * H], dt.float8e4,
                                      isOutput=False) for p in range(2)]
    wh8_d = [nc.declare_dram_parameter(f"wh8{p}", [128, 2 * 2 * H], dt.float8e4,
                                       isOutput=False) for p in range(2)]
    whh_d = nc.declare_dram_parameter("whh", [128, 4 * H], dt.bfloat16,
                                      isOutput=False)
    brow_d = nc.declare_dram_parameter("brow", [2, 512], dt.bfloat16, isOutput=False)
    sel_d = nc.declare_dram_parameter("sel", [2, 512], dt.bfloat16, isOutput=False)
    bni_d = nc.declare_dram_parameter("bni", [128, 4], dt.float32, isOutput=False)
    brown_d = nc.declare_dram_parameter("brown", [2, 256], dt.bfloat16,
                                        isOutput=False)
    h0_d = nc.declare_dram_parameter("h0", [H, BL], dt.float32, isOutput=False)
    wout_d = nc.declare_dram_parameter("wout", [128, 32], dt.bfloat16,
                                       isOutput=False)
    yout_d = nc.declare_dram_parameter("yout", [T, BL], dt.float32, isOutput=True)

    with tile.TileContext(nc) as tc:
        with (
            tc.tile_pool(name="wpool", bufs=1) as wpool,
            tc.tile_pool(name="xpool", bufs=2) as xpool,
            tc.tile_pool(name="hpool", bufs=1) as hpool,
            tc.tile_pool(name="tpool", bufs=4) as tpool,
            tc.tile_pool(name="ppool", bufs=2, space=bass.MemorySpace.PSUM) as ppool,
        ):
            # ---- persistent constants ----
            wi = [wpool.tile([128, 2, 3 * H], dt.float8e4, name=f"wi{p}",
                             tag=f"wi{p}") for p in range(2)]
            wh8 = [wpool.tile([128, 2, 2 * H], dt.float8e4, name=f"wh8{p}",
                              tag=f"wh8{p}") for p in range(2)]
            # whh: n-gate cols only [k-chunk, kc*512 + col], values * PS
            whh = wpool.tile([128, 4 * H], dt.bfloat16, name="whh", tag="whh")
            fcin = wpool.tile([FB, H], dt.bfloat16, name="fcin", tag="fcin")
            stg = wpool.tile([FB, T * BL], dt.bfloat16, name="stg", tag="stg")
            brow = wpool.tile([2, 512], dt.bfloat16, name="brow", tag="brow")
            sel = wpool.tile([2, 512], dt.bfloat16, name="sel", tag="sel")
            bni = wpool.tile([128, 4], dt.float32, name="bni", tag="bni")
            brown = wpool.tile([2, 256], dt.bfloat16, name="brown", tag="brown")
            wout = wpool.tile([128, 32], dt.bfloat16, name="wout", tag="wout")
            fcint = wpool.tile([FB, H], dt.bfloat16, name="fcint", tag="fcint")
            h_all = hpool.tile([128, 4 * BL], dt.float32, name="hall", tag="hall")
            hb = hpool.tile([128, 4, BL], dt.bfloat16, name="hb", tag="hb")
            hb8 = hpool.tile([128, 4, BL], dt.float8e4, name="hb8", tag="hb8")

            CW = SPB * BL  # columns per block

            # early DMAs: block-0 inputs + projection weights first so PE can
            # start the block-0 projection while the big weight DMAs land.
            nc.sync.dma_start(stg[:FB, 0:CW], xdat_d[:, 0:CW])
            nc.sync.dma_start(fcint[:FB, :], fcin_d[:])
            # DVE funnel: PE Matmult supports only ONE embedded sem wait, so
            # route DMA-landed matmul operands through DVE; matmul deps then
            # collapse onto the single DVE semaphore.
            nc.vector.tensor_copy(fcin[:FB, :], fcint[:FB, :])
            nc.sync.dma_start(stg[:FB, CW:2 * CW], xdat_d[:, CW:2 * CW])
            for c in range(4):
                nc.sync.dma_start(h_all[:, c * BL:(c + 1) * BL],
                                  h0_d[c * 128:(c + 1) * 128, :])
            nc.vector.tensor_copy(hb[:], h_all[:])
            nc.vector.tensor_scalar_mul(hb8[:], h_all[:], SX)
            nc.sync.dma_start(brow[:], brow_d[:])
            nc.sync.dma_start(sel[:], sel_d[:])
            nc.sync.dma_start(bni[:], bni_d[:])
            nc.sync.dma_start(brown[:], brown_d[:])
            nc.sync.dma_start(wout[:], wout_d[:])
            nc.sync.dma_start(whh[:], whh_d[:])
            for p in range(2):
                nc.sync.dma_start(wi[p][:], wi_d[p][:])
                nc.sync.dma_start(wh8[p][:], wh8_d[p][:])

            def whn(kc, c4):
                # Whh.T n-gate block: contraction chunk kc, out-chunk c4
                return whh[:, kc * H + c4 * 128: kc * H + (c4 + 1) * 128]

            def proj_col(off, xb_t, s):
                # two oc per PSUM bank, ONE accumulation group per bank,
                # then one full-bank relu evicting both chunks as scaled fp8
                for half in range(2):
                    pj = ppool.tile([128, 2 * BL], dt.float32, name="gpj",
                                    tag="gp", bufs=2)
                    for oc2 in range(2):
                        oc = half * 2 + oc2
                        nc.tensor.matmul(pj[:, oc2 * BL:(oc2 + 1) * BL],
                                         fcin[:FB, oc * 128:(oc + 1) * 128],
                                         stg[:FB, off + s * BL:off + (s + 1) * BL],
                                         start=(oc2 == 0), stop=(oc2 == 1),
                                         skip_group_check=True)
                    nc.scalar.activation(
                        xb_t[:, 2 * half:2 * half + 2, s * BL:(s + 1) * BL],
                        pj[:], AF.Relu, scale=SX)

            # block 0 projects its own inputs up front (PE filler during the
            # weight DMAs); later blocks are projected inside the prior block
            xb_cur = xpool.tile([128, 4, CW], dt.float8e4, name="xb8",
                                tag="xb8", bufs=2)
            for s in range(SPB):
                proj_col(0, xb_cur, s)

            pend = None
            for blk in range(NBLK):
                if blk + 1 < NBLK:
                    xb_next = xpool.tile([128, 4, CW], dt.float8e4, name="xb8",
                                         tag="xb8", bufs=2)
                if blk + 2 < NBLK:
                    nc.sync.dma_start(stg[:FB, (blk + 2) * CW:(blk + 3) * CW],
                                      xdat_d[:, (blk + 2) * CW:(blk + 3) * CW])
                ysb = tpool.tile([1, SPB * BL], dt.float32, name="ysb",
                                 tag="ysb", bufs=2)

                for s in range(SPB):
                    def xsl(p):
                        return xb_cur[:, 2 * p:2 * p + 2, s * BL:(s + 1) * BL]

                    # ---- x-phase: banks in the order chain(t-1) frees them
                    # r/z x-side + K=2 bias row opens each bank's group;
                    # rz banks were freed by the earliest chain events
                    # (the sigmoids), so PE restarts immediately.
                    rz_ps = []
                    for bk in range(4):  # banks: r01, r23, z01, z23
                        gp = ppool.tile([128, 2 * BL], dt.float32, name="grz",
                                        tag="grz", bufs=4)
                        nc.tensor.matmul(gp[:], brow[:, bk * 128:(bk + 1) * 128],
                                         sel[:], start=True, stop=False,
                                         skip_group_check=True)
                        for c2 in range(2):
                            m = bk * 2 + c2
                            ms = slice(m * 128, (m + 1) * 128)
                            for p in range(2):
                                nc.tensor.matmul(
                                    gp[:, c2 * BL:(c2 + 1) * BL],
                                    wi[p][:, :, ms], xsl(p),
                                    start=False, stop=False,
                                    perf_mode=DR, skip_group_check=True)
                        rz_ps.append(gp)

                    # next block's projection (gp banks, freed by rhn)
                    if blk + 1 < NBLK:
                        proj_col((blk + 1) * CW, xb_next, s)

                    # deferred output head for the previous step (gp bank)
                    if pend is not None:
                        pysb, pblk, ps_ = pend
                        yp = ppool.tile([128, 2 * BL], dt.float32, name="yp",
                                        tag="gp", bufs=2)
                        for c in range(4):
                            w0 = ps_ * 4 + c
                            nc.tensor.matmul(yp[0:1, 0:BL],
                                             wout[:, w0:w0 + 1],
                                             hb[:, c, :],
                                             start=(c == 0), stop=(c == 3),
                                             skip_group_check=True)
                        nc.scalar.activation(pysb[0:1, ps_ * BL:(ps_ + 1) * BL],
                                             yp[0:1, 0:BL], AF.Copy)
                        if ps_ == SPB - 1:
                            nc.sync.dma_start(
                                yout_d[pblk * SPB:(pblk + 1) * SPB, :],
                                pysb[0:1, :])
                        pend = None

                    # n-gate input side (gin): own banks, freed last (by sa)
                    gin_ps = []
                    for pr in range(2):
                        gp = ppool.tile([128, 2 * BL], dt.float32, name="gin",
                                        tag="gin", bufs=2)
                        for c2 in range(2):
                            c4 = pr * 2 + c2
                            ms = slice((8 + c4) * 128, (9 + c4) * 128)
                            for p in range(2):
                                nc.tensor.matmul(
                                    gp[:, c2 * BL:(c2 + 1) * BL],
                                    wi[p][:, :, ms], xsl(p),
                                    start=(c2 == 0 and p == 0),
                                    stop=(c2 == 1 and p == 1),
                                    perf_mode=DR, skip_group_check=True)
                        gin_ps.append(gp)

                    # ---- h-phase (bf16): r banks + ghn first so the chain
                    # starts early; z banks feed the late h-update
                    r_all = tpool.tile([128, 4 * BL], dt.float32, name="rall",
                                       tag="rall", bufs=2)
                    z_all = tpool.tile([128, 4 * BL], dt.float32, name="zall",
                                       tag="zall", bufs=2)
                    zp_all = tpool.tile([128, 4 * BL], dt.float32, name="zpall",
                                        tag="zpall", bufs=2)
                    ghn_ps = []

                    def rz_h(bk):
                        gp = rz_ps[bk]
                        for p in range(2):
                            for c2 in range(2):
                                m = bk * 2 + c2
                                ms = slice(m * 128, (m + 1) * 128)
                                nc.tensor.matmul(
                                    gp[:, c2 * BL:(c2 + 1) * BL],
                                    wh8[p][:, :, ms],
                                    hb8[:, 2 * p:2 * p + 2, :],
                                    start=False,
                                    stop=(c2 == 1 and p == 1),
                                    perf_mode=DR, skip_group_check=True)
                        o = (bk % 2) * 2 * BL
                        if bk < 2:
                            nc.scalar.activation(r_all[:, o:o + 2 * BL], gp[:],
                                                 AF.Sigmoid, scale=DS)
                        else:
                            nc.scalar.activation(z_all[:, o:o + 2 * BL], gp[:],
                                                 AF.Sigmoid, scale=DS)
                            nc.scalar.activation(zp_all[:, o:o + 2 * BL], gp[:],
                                                 AF.Sigmoid, scale=-DS)

                    def ghn_h(pr):
                        gp = ppool.tile([128, 2 * BL], dt.float32, name="ghn",
                                        tag="gp", bufs=2)
                        nc.tensor.matmul(gp[:], brown[:, pr * 128:(pr + 1) * 128],
                                         sel[:], start=True, stop=False,
                                         skip_group_check=True)
                        for c2 in range(2):
                            c4 = pr * 2 + c2
                            for kc in range(4):
                                nc.tensor.matmul(
                                    gp[:, c2 * BL:(c2 + 1) * BL],
                                    whn(kc, c4), hb[:, kc, :],
                                    start=False,
                                    stop=(c2 == 1 and kc == 3),
                                    skip_group_check=True)
                        ghn_ps.append(gp)

                    rz_h(0)      # r01 -> sigmoid r01
                    ghn_h(0)     # ghn01
                    rz_h(1)      # r23 -> sigmoid r23
                    ghn_h(1)     # ghn23
                    rz_h(2)      # z01
                    rz_h(3)      # z23

                    t1 = tpool.tile([128, 4 * BL], dt.float32, name="t1",
                                    tag="t1", bufs=2)
                    for c4 in range(4):
                        cs = slice(c4 * BL, (c4 + 1) * BL)
                        nc.gpsimd.tensor_mul(t1[:, cs], z_all[:, cs],
                                             h_all[:, cs])

                    # ---- chain: rhn = ghn*r (full-bank, bnh already in
                    # psum via the bias row), sa = gin + rhn (full-bank),
                    # tanh per chunk (bias bni), then the ladder-free update
                    # h' = z*h + (1-z)*n with 1-z = sigmoid(-x) from the same
                    # bank; hb recast per pair so pair0 overlaps pair1
                    sa_all = tpool.tile([128, 4 * BL], dt.float32, name="saall",
                                        tag="saall", bufs=2)
                    nsb_all = tpool.tile([128, 4 * BL], dt.float32, name="nsball",
                                         tag="nsball", bufs=2)
                    t2 = tpool.tile([128, 4 * BL], dt.float32, name="t2",
                                    tag="t2", bufs=2)
                    # all four full-bank PSUM reads lead the DVE stream so
                    # pair1's chain starts as soon as its banks stop (not
                    # queued behind pair0's elementwise tail)
                    for pr in range(2):
                        o = pr * 2 * BL
                        rhn = tpool.tile([128, 2 * BL], dt.float32, name="rhn",
                                         tag="rhn", bufs=2)
                        nc.vector.tensor_mul(rhn[:], ghn_ps[pr][:],
                                             r_all[:, o:o + 2 * BL])
                        nc.vector.tensor_add(sa_all[:, o:o + 2 * BL],
                                             gin_ps[pr][:], rhn[:])
                    for pr in range(2):
                        for c2 in range(2):
                            c4 = pr * 2 + c2
                            cs = slice(c4 * BL, (c4 + 1) * BL)
                            nc.scalar.activation(nsb_all[:, cs], sa_all[:, cs],
                                                 AF.Tanh, bias=bni[:, c4:c4 + 1],
                                                 scale=DS)
                    for pr in range(2):
                        for c2 in range(2):
                            c4 = pr * 2 + c2
                            cs = slice(c4 * BL, (c4 + 1) * BL)
                            nc.vector.tensor_mul(t2[:, cs], zp_all[:, cs],
                                                 nsb_all[:, cs])
                            nc.vector.tensor_add(h_all[:, cs], t1[:, cs],
                                                 t2[:, cs])
                        # hb8 gates the next step's rz-h matmuls: cast first
                        nc.vector.tensor_scalar_mul(
                            hb8[:, 2 * pr:2 * pr + 2, :],
                            h_all[:, pr * 2 * BL:(pr + 1) * 2 * BL], SX)
                        nc.vector.tensor_copy(hb[:, 2 * pr:2 * pr + 2, :],
                                              h_all[:, pr * 2 * BL:
                                                     (pr + 1) * 2 * BL])
                    pend = (ysb, blk, s)

                if blk + 1 < NBLK:
                    xb_cur = xb_next

            # drain the final step's head
            pysb, pblk, ps_ = pend
            yp = ppool.tile([128, 2 * BL], dt.float32, name="yp", tag="gp",
                            bufs=2)
            for c in range(4):
                w0 = ps_ * 4 + c
                nc.tensor.matmul(yp[0:1, 0:BL], wout[:, w0:w0 + 1],
                                 hb[:, c, :], start=(c == 0), stop=(c == 3),
                                 skip_group_check=True)
            nc.scalar.activation(pysb[0:1, ps_ * BL:(ps_ + 1) * BL],
                                 yp[0:1, 0:BL], AF.Copy)
            nc.sync.dma_start(yout_d[pblk * SPB:(pblk + 1) * SPB, :],
                              pysb[0:1, :])

    _legalize(nc, mybir)
    return nc


def _get_nc():
    if "nc" not in _CACHE:
        _CACHE["nc"] = _build()
    return _CACHE["nc"]


def _wsp(w):
    chunks = w.reshape(4, 128)
    out = np.zeros((128, 32), np.float32)
    for s in range(SPB):
        for c in range(4):
            out[:, s * 4 + c] = chunks[c]
    return out.astype(BF16)


def _pack_dr(wT):
    """[512, 1536] contraction-major weight -> two DoubleRow pair tensors
    [128, 2*1536] fp8: pair p rows (2p,2p+1); [k, i*1536+m] = wT[p*256+i*128+k, m]."""
    w = (wT * SW).astype(np.float32)
    out = []
    for p in range(2):
        t = np.empty((128, 2 * 3 * H), np.float32)
        for i in range(2):
            t[:, i * 3 * H:(i + 1) * 3 * H] = w[(2 * p + i) * 128:
                                                (2 * p + i + 1) * 128, :]
        out.append(np.ascontiguousarray(t).astype(F8))
    return out


def kernel(data, init, fc_in_W, fc_in_b, Wih, Whh, bih, bhh, fc_out_W, fc_out_b):
    from concourse.bass_utils import run_bass_kernel_spmd

    data = np.asarray(data, np.float32)
    init = np.asarray(init, np.float32)
    fc_in_W = np.asarray(fc_in_W, np.float32)
    fc_in_b = np.asarray(fc_in_b, np.float32)
    Wih = np.asarray(Wih, np.float32)
    Whh = np.asarray(Whh, np.float32)
    bih = np.asarray(bih, np.float32)
    bhh = np.asarray(bhh, np.float32)
    fc_out_W = np.asarray(fc_out_W, np.float32)
    fc_out_b = np.asarray(fc_out_b, np.float32)

    nc = _get_nc()

    sel = np.zeros((2, 512), np.float32)
    sel[0, :256] = 1.0
    sel[1, 256:] = 1.0
    sel = sel.astype(BF16)

    in_maps = []
    for i in range(8):
        k, j = i // 2, i % 2
        d = data[j * BL:(j + 1) * BL]            # [256, 64, 64] (b,f,t)
        if k >= 2:
            d = d[:, :, ::-1]                    # reversed-time branches
        xdat = np.ascontiguousarray(d.transpose(1, 2, 0)).reshape(F, T * BL)
        xdat = np.concatenate([xdat, np.ones((1, T * BL), np.float32)], axis=0)
        fcin = np.concatenate([fc_in_W[k].T, fc_in_b[k][None, :]], axis=0)
        brz = (bih[k][:2 * H] + bhh[k][:2 * H]) * PS     # [1024]
        brow = brz.reshape(8, 128)               # chunk-major
        brow2 = np.empty((2, 512), np.float32)
        for bk in range(4):
            for i2 in range(2):
                brow2[i2, bk * 128:(bk + 1) * 128] = brow[2 * bk + i2]
        bnhr = (bhh[k][2 * H:] * PS).reshape(4, 128)
        brown_n = np.empty((2, 256), np.float32)
        for pr in range(2):
            for i2 in range(2):
                brown_n[i2, pr * 128:(pr + 1) * 128] = bnhr[2 * pr + i2]
        brown_n = np.ascontiguousarray(brown_n).astype(BF16)
        wip = _pack_dr(Wih[k].T)
        whT_rz = (Whh[k].T[:, :2 * H] * SW).astype(np.float32)  # [512, 1024]
        wh8p = []
        for p in range(2):
            t = np.empty((128, 2 * 2 * H), np.float32)
            for i2 in range(2):
                t[:, i2 * 2 * H:(i2 + 1) * 2 * H] = whT_rz[(2 * p + i2) * 128:
                                                           (2 * p + i2 + 1) * 128]
            wh8p.append(np.ascontiguousarray(t).astype(F8))
        # whh: n cols only, [128, kc*512 + m] = Whh.T[kc*128+kk, 2H+m] * PS
        whT = (Whh[k].T[:, 2 * H:] * PS).astype(np.float32)  # [512, 512]
        whh = np.empty((128, 4 * H), np.float32)
        for kc in range(4):
            whh[:, kc * H:(kc + 1) * H] = whT[kc * 128:(kc + 1) * 128]
        in_maps.append({
            "xdat": np.ascontiguousarray(xdat).astype(BF16),
            "fcin": np.ascontiguousarray(fcin).astype(BF16),  # [65, 512]
            "wi0": wip[0], "wi1": wip[1],
            "wh80": wh8p[0], "wh81": wh8p[1],
            "whh": np.ascontiguousarray(whh).astype(BF16),
            "brow": np.ascontiguousarray(brow2).astype(BF16),
            "sel": sel,
            "bni": np.ascontiguousarray(bih[k][2 * H:].reshape(4, 128).T),
            "brown": brown_n,
            "h0": np.ascontiguousarray(init[j * BL:(j + 1) * BL].T),
            "wout": _wsp(fc_out_W[k % 2]),
        })

    kw = {"trace": True} if TRACE else {}
    res = run_bass_kernel_spmd(nc, in_maps, list(range(8)), **kw)
    LAST["res"] = res
    y = [np.asarray(res.results[i]["yout"], np.float32) for i in range(8)]

    air_out = np.empty((B, T), np.float32)
    bed_out = np.empty((B, T), np.float32)
    for j in range(2):
        sl = slice(j * BL, (j + 1) * BL)
        air_out[sl] = (y[0 + j] + y[4 + j][::-1]).T + fc_out_b[0]
        bed_out[sl] = (y[2 + j] + y[6 + j][::-1]).T + fc_out_b[1]
    return air_out, bed_out


# revision 25
# speedup vs baseline: 1.0574x; 1.0084x over previous
"""4-branch bidirectional GRU (nn_RNN_2817498546846) on 8 TRN2 NeuronCores.

Sharding: core i handles cell k=i//2 (air0,bed0,air1,bed1) and batch half
j=i%2 (256 rows). Weights per-core = one cell only; no collectives.
Cells 2,3 consume the time-reversed sequence -> host reverses their data,
so the device program is identical on all cores (pure SPMD).

Mixed-precision edition: the x-side gate matmuls (r/z/n input projections
against Wih) run as fp8e4m3 DoubleRow matmuls (2 contraction rows/cycle,
x scaled by 16, Wih by 2048); the h-side matmuls (Whh) stay bf16 for
recurrence accuracy, with Whh pre-scaled by 2^15 on the host so both
sides accumulate in the same PSUM domain (descaled at the activations).
r/z biases enter PSUM via one K=2 bias-row matmul per bank so each r/z
bank is consumed by ONE full-bank [128,512] sigmoid; full-bank PSUM reads
also make bank recycling WAR-sound (start=True wipes its whole bank).
Step emission order is arranged so the x-phase of step t+1 reuses banks
in the order the chain of step t releases them (rz first, ghn/proj mid,
gin last), keeping PE warm through the serial chain.
"""

import sys
import numpy as np

sys.path.insert(0, "/opt/trn_rl_repo")

import ml_dtypes

B, F, T, H, K = 512, 64, 64, 512, 4
BL = 256          # batch per core
NBLK, SPB = 8, 8  # 8 blocks x 8 steps
BF16 = ml_dtypes.bfloat16
F8 = ml_dtypes.float8_e4m3   # TRN fp8e4 (IEEE-ish, max 240)

SX = 16.0      # x quantization scale
SW = 2048.0    # Wih quantization scale
PS = SX * SW   # psum scale (2^15)
DS = 1.0 / PS

_CACHE = {}
TRACE = False   # test harness sets True to capture NTFF profile
LAST = {}       # stashes the BassKernelResults of the most recent run


def _legalize(nc, mybir):
    """Walrus codegen allows at most ONE embedded sem wait per instruction
    (libwalrus setupSyncWait asserts count==1 for every ISA struct). Engines
    execute their streams in order, so extra waits move onto same-engine
    NoOps inserted immediately before the offending instruction."""
    n_split = 0
    for f in nc.m.functions:
        for b in f.blocks:
            insts = b.instructions
            out = []
            for ins in insts:
                si = getattr(ins, "sync_info", None)
                waits = list(si.on_wait) if si is not None and si.on_wait else []
                if len(waits) > 1:
                    for k, w in enumerate(waits[:-1]):
                        nop = mybir.InstNoOp(
                            name=f"{ins.name}-lw{k}",
                            engine=ins.engine,
                            bass_nofuse=True,
                            sync_info=mybir.SyncInfo(on_wait=[w], on_update=[]),
                        )
                        out.append(nop)
                        n_split += 1
                    ups = list(si.on_update) if si.on_update else []
                    ins.sync_info = mybir.SyncInfo(on_wait=[waits[-1]], on_update=ups)
                out.append(ins)
            insts[:] = out
    return n_split


def _build():
    import concourse.bass as bass
    import concourse.tile as tile
    from concourse import mybir

    dt = mybir.dt
    AF = mybir.ActivationFunctionType
    OP = mybir.AluOpType
    DR = mybir.MatmulPerfMode.DoubleRow

    nc = bass.Bass("TRN2", target_bir_lowering=False, debug=False, num_devices=8)

    FB = F + 1  # input feature rows + ones row (carries fc_in bias)
    xdat_d = nc.declare_dram_parameter("xdat", [FB, T * BL], dt.bfloat16,
                                       isOutput=False)
    fcin_d = nc.declare_dram_parameter("fcin", [FB, H], dt.bfloat16, isOutput=False)
    wi_d = [nc.declare_dram_parameter(f"wi{p}", [128, 2 * 3 * H], dt.float8e4,
                                      isOutput=False) for p in range(2)]
    wh8_d = [nc.declare_dram_parameter(f"wh8{p}", [128, 2 * 2 * H], dt.float8e4,
                                       isOutput=False) for p in range(2)]
    whh_d = nc.declare_dram_parameter("whh", [128, 4 * H], dt.bfloat16,
                                      isOutput=False)
    whn8_d = [nc.declare_dram_parameter(f"whn8{p}", [128, 2 * H], dt.float8e4,
                                        isOutput=False) for p in range(2)]
    brow_d = nc.declare_dram_parameter("brow", [2, 512], dt.bfloat16, isOutput=False)
    sel_d = nc.declare_dram_parameter("sel", [2, 512], dt.bfloat16, isOutput=False)
    bni_d = nc.declare_dram_parameter("bni", [128, 4], dt.float32, isOutput=False)
    brown_d = nc.declare_dram_parameter("brown", [2, 256], dt.bfloat16,
                                        isOutput=False)
    h0_d = nc.declare_dram_parameter("h0", [H, BL], dt.float32, isOutput=False)
    wout_d = nc.declare_dram_parameter("wout", [128, 32], dt.bfloat16,
                                       isOutput=False)
    yout_d = nc.declare_dram_parameter("yout", [T, BL], dt.float32, isOutput=True)

    with tile.TileContext(nc) as tc:
        with (
            tc.tile_pool(name="wpool", bufs=1) as wpool,
            tc.tile_pool(name="xpool", bufs=2) as xpool,
            tc.tile_pool(name="hpool", bufs=1) as hpool,
            tc.tile_pool(name="tpool", bufs=4) as tpool,
            tc.tile_pool(name="ppool", bufs=2, space=bass.MemorySpace.PSUM) as ppool,
        ):
            # ---- persistent constants ----
            wi = [wpool.tile([128, 2, 3 * H], dt.float8e4, name=f"wi{p}",
                             tag=f"wi{p}") for p in range(2)]
            wh8 = [wpool.tile([128, 2, 2 * H], dt.float8e4, name=f"wh8{p}",
                              tag=f"wh8{p}") for p in range(2)]
            # whh: n-gate cols only [k-chunk, kc*512 + col], values * PS
            whh = wpool.tile([128, 4 * H], dt.bfloat16, name="whh", tag="whh")
            whn8 = [wpool.tile([128, 2, H], dt.float8e4, name=f"whn8{p}",
                               tag=f"whn8{p}") for p in range(2)]
            fcin = wpool.tile([FB, H], dt.bfloat16, name="fcin", tag="fcin")
            stg = wpool.tile([FB, T * BL], dt.bfloat16, name="stg", tag="stg")
            brow = wpool.tile([2, 512], dt.bfloat16, name="brow", tag="brow")
            sel = wpool.tile([2, 512], dt.bfloat16, name="sel", tag="sel")
            bni = wpool.tile([128, 4], dt.float32, name="bni", tag="bni")
            brown = wpool.tile([2, 256], dt.bfloat16, name="brown", tag="brown")
            wout = wpool.tile([128, 32], dt.bfloat16, name="wout", tag="wout")
            fcint = wpool.tile([FB, H], dt.bfloat16, name="fcint", tag="fcint")
            h_all = hpool.tile([128, 4 * BL], dt.float32, name="hall", tag="hall")
            hb = hpool.tile([128, 4, BL], dt.bfloat16, name="hb", tag="hb")
            hb8 = hpool.tile([128, 4, BL], dt.float8e4, name="hb8", tag="hb8")

            CW = SPB * BL  # columns per block

            # early DMAs: block-0 inputs + projection weights first so PE can
            # start the block-0 projection while the big weight DMAs land.
            nc.sync.dma_start(stg[:FB, 0:CW], xdat_d[:, 0:CW])
            nc.sync.dma_start(fcint[:FB, :], fcin_d[:])
            # DVE funnel: PE Matmult supports only ONE embedded sem wait, so
            # route DMA-landed matmul operands through DVE; matmul deps then
            # collapse onto the single DVE semaphore.
            nc.vector.tensor_copy(fcin[:FB, :], fcint[:FB, :])
            nc.sync.dma_start(stg[:FB, CW:2 * CW], xdat_d[:, CW:2 * CW])
            for c in range(4):
                nc.sync.dma_start(h_all[:, c * BL:(c + 1) * BL],
                                  h0_d[c * 128:(c + 1) * 128, :])
            nc.vector.tensor_copy(hb[:], h_all[:])
            nc.vector.tensor_scalar_mul(hb8[:], h_all[:], SX)
            nc.sync.dma_start(brow[:], brow_d[:])
            nc.sync.dma_start(sel[:], sel_d[:])
            nc.sync.dma_start(bni[:], bni_d[:])
            nc.sync.dma_start(brown[:], brown_d[:])
            nc.sync.dma_start(wout[:], wout_d[:])
            nc.sync.dma_start(whh[:], whh_d[:])
            for p in range(2):
                nc.sync.dma_start(wi[p][:], wi_d[p][:])
                nc.sync.dma_start(wh8[p][:], wh8_d[p][:])
                nc.sync.dma_start(whn8[p][:], whn8_d[p][:])

            def whn(kc, c4):
                # Whh.T n-gate block: contraction chunk kc, out-chunk c4
                return whh[:, kc * H + c4 * 128: kc * H + (c4 + 1) * 128]

            def proj_col(off, xb_t, s):
                # two oc per PSUM bank, ONE accumulation group per bank,
                # then one full-bank relu evicting both chunks as scaled fp8
                for half in range(2):
                    pj = ppool.tile([128, 2 * BL], dt.float32, name="gpj",
                                    tag="gp", bufs=2)
                    for oc2 in range(2):
                        oc = half * 2 + oc2
                        nc.tensor.matmul(pj[:, oc2 * BL:(oc2 + 1) * BL],
                                         fcin[:FB, oc * 128:(oc + 1) * 128],
                                         stg[:FB, off + s * BL:off + (s + 1) * BL],
                                         start=(oc2 == 0), stop=(oc2 == 1),
                                         skip_group_check=True)
                    nc.scalar.activation(
                        xb_t[:, 2 * half:2 * half + 2, s * BL:(s + 1) * BL],
                        pj[:], AF.Relu, scale=SX)

            # block 0 projects its own inputs up front (PE filler during the
            # weight DMAs); later blocks are projected inside the prior block
            xb_cur = xpool.tile([128, 4, CW], dt.float8e4, name="xb8",
                                tag="xb8", bufs=2)
            for s in range(SPB):
                proj_col(0, xb_cur, s)

            pend = None
            for blk in range(NBLK):
                if blk + 1 < NBLK:
                    xb_next = xpool.tile([128, 4, CW], dt.float8e4, name="xb8",
                                         tag="xb8", bufs=2)
                if blk + 2 < NBLK:
                    nc.sync.dma_start(stg[:FB, (blk + 2) * CW:(blk + 3) * CW],
                                      xdat_d[:, (blk + 2) * CW:(blk + 3) * CW])
                ysb = tpool.tile([1, SPB * BL], dt.float32, name="ysb",
                                 tag="ysb", bufs=2)

                for s in range(SPB):
                    def xsl(p):
                        return xb_cur[:, 2 * p:2 * p + 2, s * BL:(s + 1) * BL]

                    # ---- x-phase: banks in the order chain(t-1) frees them
                    # r/z x-side + K=2 bias row opens each bank's group;
                    # rz banks were freed by the earliest chain events
                    # (the sigmoids), so PE restarts immediately.
                    rz_ps = []
                    for bk in range(4):  # banks: r01, r23, z01, z23
                        gp = ppool.tile([128, 2 * BL], dt.float32, name="grz",
                                        tag="grz", bufs=4)
                        nc.tensor.matmul(gp[:], brow[:, bk * 128:(bk + 1) * 128],
                                         sel[:], start=True, stop=False,
                                         skip_group_check=True)
                        for c2 in range(2):
                            m = bk * 2 + c2
                            ms = slice(m * 128, (m + 1) * 128)
                            for p in range(2):
                                nc.tensor.matmul(
                                    gp[:, c2 * BL:(c2 + 1) * BL],
                                    wi[p][:, :, ms], xsl(p),
                                    start=False, stop=False,
                                    perf_mode=DR, skip_group_check=True)
                        rz_ps.append(gp)

                    # next block's projection (gp banks, freed by rhn)
                    if blk + 1 < NBLK:
                        proj_col((blk + 1) * CW, xb_next, s)

                    # deferred output head for the previous step (gp bank)
                    if pend is not None:
                        pysb, pblk, ps_ = pend
                        yp = ppool.tile([128, 2 * BL], dt.float32, name="yp",
                                        tag="gp", bufs=2)
                        for c in range(4):
                            w0 = ps_ * 4 + c
                            nc.tensor.matmul(yp[0:1, 0:BL],
                                             wout[:, w0:w0 + 1],
                                             hb[:, c, :],
                                             start=(c == 0), stop=(c == 3),
                                             skip_group_check=True)
                        nc.scalar.activation(pysb[0:1, ps_ * BL:(ps_ + 1) * BL],
                                             yp[0:1, 0:BL], AF.Copy)
                        if ps_ == SPB - 1:
                            nc.sync.dma_start(
                                yout_d[pblk * SPB:(pblk + 1) * SPB, :],
                                pysb[0:1, :])
                        pend = None

                    # n-gate input side (gin): own banks, freed last (by sa)
                    gin_ps = []
                    for pr in range(2):
                        gp = ppool.tile([128, 2 * BL], dt.float32, name="gin",
                                        tag="gin", bufs=2)
                        for c2 in range(2):
                            c4 = pr * 2 + c2
                            ms = slice((8 + c4) * 128, (9 + c4) * 128)
                            for p in range(2):
                                nc.tensor.matmul(
                                    gp[:, c2 * BL:(c2 + 1) * BL],
                                    wi[p][:, :, ms], xsl(p),
                                    start=(c2 == 0 and p == 0),
                                    stop=(c2 == 1 and p == 1),
                                    perf_mode=DR, skip_group_check=True)
                        gin_ps.append(gp)

                    # ---- h-phase (bf16): r banks + ghn first so the chain
                    # starts early; z banks feed the late h-update
                    r_all = tpool.tile([128, 4 * BL], dt.float32, name="rall",
                                       tag="rall", bufs=2)
                    z_all = tpool.tile([128, 4 * BL], dt.float32, name="zall",
                                       tag="zall", bufs=2)
                    zp_all = tpool.tile([128, 4 * BL], dt.float32, name="zpall",
                                        tag="zpall", bufs=2)
                    ghn_ps = []

                    def rz_h(bk):
                        gp = rz_ps[bk]
                        for p in range(2):
                            for c2 in range(2):
                                m = bk * 2 + c2
                                ms = slice(m * 128, (m + 1) * 128)
                                nc.tensor.matmul(
                                    gp[:, c2 * BL:(c2 + 1) * BL],
                                    wh8[p][:, :, ms],
                                    hb8[:, 2 * p:2 * p + 2, :],
                                    start=False,
                                    stop=(c2 == 1 and p == 1),
                                    perf_mode=DR, skip_group_check=True)
                        o = (bk % 2) * 2 * BL
                        if bk < 2:
                            nc.scalar.activation(r_all[:, o:o + 2 * BL], gp[:],
                                                 AF.Sigmoid, scale=DS)
                        else:
                            nc.scalar.activation(z_all[:, o:o + 2 * BL], gp[:],
                                                 AF.Sigmoid, scale=DS)
                            nc.scalar.activation(zp_all[:, o:o + 2 * BL], gp[:],
                                                 AF.Sigmoid, scale=-DS)

                    def ghn_h(pr):
                        gp = ppool.tile([128, 2 * BL], dt.float32, name="ghn",
                                        tag="gp", bufs=2)
                        nc.tensor.matmul(gp[:], brown[:, pr * 128:(pr + 1) * 128],
                                         sel[:], start=True, stop=False,
                                         skip_group_check=True)
                        for c2 in range(2):
                            c4 = pr * 2 + c2
                            ms = slice(c4 * 128, (c4 + 1) * 128)
                            for p in range(2):
                                nc.tensor.matmul(
                                    gp[:, c2 * BL:(c2 + 1) * BL],
                                    whn8[p][:, :, ms],
                                    hb8[:, 2 * p:2 * p + 2, :],
                                    start=False,
                                    stop=(c2 == 1 and p == 1),
                                    perf_mode=DR, skip_group_check=True)
                        ghn_ps.append(gp)

                    rz_h(0)      # r01 -> sigmoid r01
                    ghn_h(0)     # ghn01
                    rz_h(1)      # r23 -> sigmoid r23
                    ghn_h(1)     # ghn23
                    rz_h(2)      # z01
                    rz_h(3)      # z23

                    t1 = tpool.tile([128, 4 * BL], dt.float32, name="t1",
                                    tag="t1", bufs=2)
                    for c4 in range(4):
                        cs = slice(c4 * BL, (c4 + 1) * BL)
                        nc.gpsimd.tensor_mul(t1[:, cs], z_all[:, cs],
                                             h_all[:, cs])

                    # ---- chain: rhn = ghn*r (full-bank, bnh already in
                    # psum via the bias row), sa = gin + rhn (full-bank),
                    # tanh per chunk (bias bni), then the ladder-free update
                    # h' = z*h + (1-z)*n with 1-z = sigmoid(-x) from the same
                    # bank; hb recast per pair so pair0 overlaps pair1
                    sa_all = tpool.tile([128, 4 * BL], dt.float32, name="saall",
                                        tag="saall", bufs=2)
                    nsb_all = tpool.tile([128, 4 * BL], dt.float32, name="nsball",
                                         tag="nsball", bufs=2)
                    t2 = tpool.tile([128, 4 * BL], dt.float32, name="t2",
                                    tag="t2", bufs=2)
                    # all four full-bank PSUM reads lead the DVE stream so
                    # pair1's chain starts as soon as its banks stop (not
                    # queued behind pair0's elementwise tail)
                    for pr in range(2):
                        o = pr * 2 * BL
                        rhn = tpool.tile([128, 2 * BL], dt.float32, name="rhn",
                                         tag="rhn", bufs=2)
                        nc.vector.tensor_mul(rhn[:], ghn_ps[pr][:],
                                             r_all[:, o:o + 2 * BL])
                        nc.vector.tensor_add(sa_all[:, o:o + 2 * BL],
                                             gin_ps[pr][:], rhn[:])
                    for pr in range(2):
                        for c2 in range(2):
                            c4 = pr * 2 + c2
                            cs = slice(c4 * BL, (c4 + 1) * BL)
                            nc.scalar.activation(nsb_all[:, cs], sa_all[:, cs],
                                                 AF.Tanh, bias=bni[:, c4:c4 + 1],
                                                 scale=DS)
                    for pr in range(2):
                        for c2 in range(2):
                            c4 = pr * 2 + c2
                            cs = slice(c4 * BL, (c4 + 1) * BL)
                            nc.vector.tensor_mul(t2[:, cs], zp_all[:, cs],
                                                 nsb_all[:, cs])
                            nc.vector.tensor_add(h_all[:, cs], t1[:, cs],
                                                 t2[:, cs])
                        # hb8 gates the next step's rz-h matmuls: cast first
                        nc.vector.tensor_scalar_mul(
                            hb8[:, 2 * pr:2 * pr + 2, :],
                            h_all[:, pr * 2 * BL:(pr + 1) * 2 * BL], SX)
                        nc.vector.tensor_copy(hb[:, 2 * pr:2 * pr + 2, :],
                                              h_all[:, pr * 2 * BL:
                                                     (pr + 1) * 2 * BL])
                    pend = (ysb, blk, s)

                if blk + 1 < NBLK:
                    xb_cur = xb_next

            # drain the final step's head
            pysb, pblk, ps_ = pend
            yp = ppool.tile([128, 2 * BL], dt.float32, name="yp", tag="gp",
                            bufs=2)
            for c in range(4):
                w0 = ps_ * 4 + c
                nc.tensor.matmul(yp[0:1, 0:BL], wout[:, w0:w0 + 1],
                                 hb[:, c, :], start=(c == 0), stop=(c == 3),
                                 skip_group_check=True)
            nc.scalar.activation(pysb[0:1, ps_ * BL:(ps_ + 1) * BL],
                                 yp[0:1, 0:BL], AF.Copy)
            nc.sync.dma_start(yout_d[pblk * SPB:(pblk + 1) * SPB, :],
                              pysb[0:1, :])

    _legalize(nc, mybir)
    return nc


def _get_nc():
    if "nc" not in _CACHE:
        _CACHE["nc"] = _build()
    return _CACHE["nc"]


def _wsp(w):
    chunks = w.reshape(4, 128)
    out = np.zeros((128, 32), np.float32)
    for s in range(SPB):
        for c in range(4):
            out[:, s * 4 + c] = chunks[c]
    return out.astype(BF16)


def _pack_dr(wT):
    """[512, 1536] contraction-major weight -> two DoubleRow pair tensors
    [128, 2*1536] fp8: pair p rows (2p,2p+1); [k, i*1536+m] = wT[p*256+i*128+k, m]."""
    w = (wT * SW).astype(np.float32)
    out = []
    for p in range(2):
        t = np.empty((128, 2 * 3 * H), np.float32)
        for i in range(2):
            t[:, i * 3 * H:(i + 1) * 3 * H] = w[(2 * p + i) * 128:
                                                (2 * p + i + 1) * 128, :]
        out.append(np.ascontiguousarray(t).astype(F8))
    return out


def kernel(data, init, fc_in_W, fc_in_b, Wih, Whh, bih, bhh, fc_out_W, fc_out_b):
    from concourse.bass_utils import run_bass_kernel_spmd

    data = np.asarray(data, np.float32)
    init = np.asarray(init, np.float32)
    fc_in_W = np.asarray(fc_in_W, np.float32)
    fc_in_b = np.asarray(fc_in_b, np.float32)
    Wih = np.asarray(Wih, np.float32)
    Whh = np.asarray(Whh, np.float32)
    bih = np.asarray(bih, np.float32)
    bhh = np.asarray(bhh, np.float32)
    fc_out_W = np.asarray(fc_out_W, np.float32)
    fc_out_b = np.asarray(fc_out_b, np.float32)

    nc = _get_nc()

    sel = np.zeros((2, 512), np.float32)
    sel[0, :256] = 1.0
    sel[1, 256:] = 1.0
    sel = sel.astype(BF16)

    in_maps = []
    for i in range(8):
        k, j = i // 2, i % 2
        d = data[j * BL:(j + 1) * BL]            # [256, 64, 64] (b,f,t)
        if k >= 2:
            d = d[:, :, ::-1]                    # reversed-time branches
        xdat = np.ascontiguousarray(d.transpose(1, 2, 0)).reshape(F, T * BL)
        xdat = np.concatenate([xdat, np.ones((1, T * BL), np.float32)], axis=0)
        fcin = np.concatenate([fc_in_W[k].T, fc_in_b[k][None, :]], axis=0)
        brz = (bih[k][:2 * H] + bhh[k][:2 * H]) * PS     # [1024]
        brow = brz.reshape(8, 128)               # chunk-major
        brow2 = np.empty((2, 512), np.float32)
        for bk in range(4):
            for i2 in range(2):
                brow2[i2, bk * 128:(bk + 1) * 128] = brow[2 * bk + i2]
        bnhr = (bhh[k][2 * H:] * PS).reshape(4, 128)
        brown_n = np.empty((2, 256), np.float32)
        for pr in range(2):
            for i2 in range(2):
                brown_n[i2, pr * 128:(pr + 1) * 128] = bnhr[2 * pr + i2]
        brown_n = np.ascontiguousarray(brown_n).astype(BF16)
        wip = _pack_dr(Wih[k].T)
        whT_rz = (Whh[k].T[:, :2 * H] * SW).astype(np.float32)  # [512, 1024]
        wh8p = []
        for p in range(2):
            t = np.empty((128, 2 * 2 * H), np.float32)
            for i2 in range(2):
                t[:, i2 * 2 * H:(i2 + 1) * 2 * H] = whT_rz[(2 * p + i2) * 128:
                                                           (2 * p + i2 + 1) * 128]
            wh8p.append(np.ascontiguousarray(t).astype(F8))
        whT_n = (Whh[k].T[:, 2 * H:] * SW).astype(np.float32)  # [512, 512]
        whn8p = []
        for p in range(2):
            t = np.empty((128, 2 * H), np.float32)
            for i2 in range(2):
                t[:, i2 * H:(i2 + 1) * H] = whT_n[(2 * p + i2) * 128:
                                                  (2 * p + i2 + 1) * 128]
            whn8p.append(np.ascontiguousarray(t).astype(F8))
        # whh: n cols only, [128, kc*512 + m] = Whh.T[kc*128+kk, 2H+m] * PS
        whT = (Whh[k].T[:, 2 * H:] * PS).astype(np.float32)  # [512, 512]
        whh = np.empty((128, 4 * H), np.float32)
        for kc in range(4):
            whh[:, kc * H:(kc + 1) * H] = whT[kc * 128:(kc + 1) * 128]
        in_maps.append({
            "xdat": np.ascontiguousarray(xdat).astype(BF16),
            "fcin": np.ascontiguousarray(fcin).astype(BF16),  # [65, 512]
            "wi0": wip[0], "wi1": wip[1],
            "wh80": wh8p[0], "wh81": wh8p[1],
            "whn80": whn8p[0], "whn81": whn8p[1],
            "whh": np.ascontiguousarray(whh).astype(BF16),
            "brow": np.ascontiguousarray(brow2).astype(BF16),
            "sel": sel,
            "bni": np.ascontiguousarray(bih[k][2 * H:].reshape(4, 128).T),
            "brown": brown_n,
            "h0": np.ascontiguousarray(init[j * BL:(j + 1) * BL].T),
            "wout": _wsp(fc_out_W[k % 2]),
        })

    kw = {"trace": True} if TRACE else {}
    res = run_bass_kernel_spmd(nc, in_maps, list(range(8)), **kw)
    LAST["res"] = res
    y = [np.asarray(res.results[i]["yout"], np.float32) for i in range(8)]

    air_out = np.empty((B, T), np.float32)
    bed_out = np.empty((B, T), np.float32)
    for j in range(2):
        sl = slice(j * BL, (j + 1) * BL)
        air_out[sl] = (y[0 + j] + y[4 + j][::-1]).T + fc_out_b[0]
        bed_out[sl] = (y[2 + j] + y[6 + j][::-1]).T + fc_out_b[1]
    return air_out, bed_out


# revision 26
# speedup vs baseline: 1.0653x; 1.0074x over previous
"""4-branch bidirectional GRU (nn_RNN_2817498546846) on 8 TRN2 NeuronCores.

Sharding: core i handles cell k=i//2 (air0,bed0,air1,bed1) and batch half
j=i%2 (256 rows). Weights per-core = one cell only; no collectives.
Cells 2,3 consume the time-reversed sequence -> host reverses their data,
so the device program is identical on all cores (pure SPMD).

Mixed-precision edition: the x-side gate matmuls (r/z/n input projections
against Wih) run as fp8e4m3 DoubleRow matmuls (2 contraction rows/cycle,
x scaled by 16, Wih by 2048); the h-side matmuls (Whh) stay bf16 for
recurrence accuracy, with Whh pre-scaled by 2^15 on the host so both
sides accumulate in the same PSUM domain (descaled at the activations).
r/z biases enter PSUM via one K=2 bias-row matmul per bank so each r/z
bank is consumed by ONE full-bank [128,512] sigmoid; full-bank PSUM reads
also make bank recycling WAR-sound (start=True wipes its whole bank).
Step emission order is arranged so the x-phase of step t+1 reuses banks
in the order the chain of step t releases them (rz first, ghn/proj mid,
gin last), keeping PE warm through the serial chain.
"""

import sys
import numpy as np

sys.path.insert(0, "/opt/trn_rl_repo")

import ml_dtypes

B, F, T, H, K = 512, 64, 64, 512, 4
BL = 256          # batch per core
NBLK, SPB = 8, 8  # 8 blocks x 8 steps
BF16 = ml_dtypes.bfloat16
F8 = ml_dtypes.float8_e4m3   # TRN fp8e4 (IEEE-ish, max 240)

SX = 16.0      # x quantization scale
SW = 2048.0    # Wih quantization scale
PS = SX * SW   # psum scale (2^15)
DS = 1.0 / PS

_CACHE = {}
TRACE = False   # test harness sets True to capture NTFF profile
LAST = {}       # stashes the BassKernelResults of the most recent run


def _legalize(nc, mybir):
    """Walrus codegen allows at most ONE embedded sem wait per instruction
    (libwalrus setupSyncWait asserts count==1 for every ISA struct). Engines
    execute their streams in order, so extra waits move onto same-engine
    NoOps inserted immediately before the offending instruction."""
    n_split = 0
    for f in nc.m.functions:
        for b in f.blocks:
            insts = b.instructions
            out = []
            for ins in insts:
                si = getattr(ins, "sync_info", None)
                waits = list(si.on_wait) if si is not None and si.on_wait else []
                if len(waits) > 1:
                    for k, w in enumerate(waits[:-1]):
                        nop = mybir.InstNoOp(
                            name=f"{ins.name}-lw{k}",
                            engine=ins.engine,
                            bass_nofuse=True,
                            sync_info=mybir.SyncInfo(on_wait=[w], on_update=[]),
                        )
                        out.append(nop)
                        n_split += 1
                    ups = list(si.on_update) if si.on_update else []
                    ins.sync_info = mybir.SyncInfo(on_wait=[waits[-1]], on_update=ups)
                out.append(ins)
            insts[:] = out
    return n_split


def _build():
    import concourse.bass as bass
    import concourse.tile as tile
    from concourse import mybir

    dt = mybir.dt
    AF = mybir.ActivationFunctionType
    OP = mybir.AluOpType
    DR = mybir.MatmulPerfMode.DoubleRow

    nc = bass.Bass("TRN2", target_bir_lowering=False, debug=False, num_devices=8)

    FB = F + 1  # input feature rows + ones row (carries fc_in bias)
    xdat_d = nc.declare_dram_parameter("xdat", [FB, T * BL], dt.bfloat16,
                                       isOutput=False)
    fcin_d = nc.declare_dram_parameter("fcin", [FB, H], dt.bfloat16, isOutput=False)
    wi_d = [nc.declare_dram_parameter(f"wi{p}", [128, 2 * 3 * H], dt.float8e4,
                                      isOutput=False) for p in range(2)]
    wh8_d = [nc.declare_dram_parameter(f"wh8{p}", [128, 2 * 2 * H], dt.float8e4,
                                       isOutput=False) for p in range(2)]
    whh_d = nc.declare_dram_parameter("whh", [128, 4 * H], dt.bfloat16,
                                      isOutput=False)
    whn8_d = [nc.declare_dram_parameter(f"whn8{p}", [128, 2 * H], dt.float8e4,
                                        isOutput=False) for p in range(2)]
    brow_d = nc.declare_dram_parameter("brow", [2, 512], dt.bfloat16, isOutput=False)
    sel_d = nc.declare_dram_parameter("sel", [2, 512], dt.bfloat16, isOutput=False)
    bni_d = nc.declare_dram_parameter("bni", [128, 4], dt.float32, isOutput=False)
    brown_d = nc.declare_dram_parameter("brown", [2, 256], dt.bfloat16,
                                        isOutput=False)
    h0_d = nc.declare_dram_parameter("h0", [H, BL], dt.float32, isOutput=False)
    wout_d = nc.declare_dram_parameter("wout", [128, 32], dt.bfloat16,
                                       isOutput=False)
    yout_d = nc.declare_dram_parameter("yout", [T, BL], dt.float32, isOutput=True)

    with tile.TileContext(nc) as tc:
        with (
            tc.tile_pool(name="wpool", bufs=1) as wpool,
            tc.tile_pool(name="xpool", bufs=2) as xpool,
            tc.tile_pool(name="hpool", bufs=1) as hpool,
            tc.tile_pool(name="tpool", bufs=4) as tpool,
            tc.tile_pool(name="ppool", bufs=2, space=bass.MemorySpace.PSUM) as ppool,
        ):
            # ---- persistent constants ----
            wi = [wpool.tile([128, 2, 3 * H], dt.float8e4, name=f"wi{p}",
                             tag=f"wi{p}") for p in range(2)]
            wh8 = [wpool.tile([128, 2, 2 * H], dt.float8e4, name=f"wh8{p}",
                              tag=f"wh8{p}") for p in range(2)]
            # whh: n-gate cols only [k-chunk, kc*512 + col], values * PS
            whh = wpool.tile([128, 4 * H], dt.bfloat16, name="whh", tag="whh")
            whn8 = [wpool.tile([128, 2, H], dt.float8e4, name=f"whn8{p}",
                               tag=f"whn8{p}") for p in range(2)]
            fcin = wpool.tile([FB, H], dt.bfloat16, name="fcin", tag="fcin")
            stg = wpool.tile([FB, T * BL], dt.bfloat16, name="stg", tag="stg")
            brow = wpool.tile([2, 512], dt.bfloat16, name="brow", tag="brow")
            sel = wpool.tile([2, 512], dt.bfloat16, name="sel", tag="sel")
            bni = wpool.tile([128, 4], dt.float32, name="bni", tag="bni")
            brown = wpool.tile([2, 256], dt.bfloat16, name="brown", tag="brown")
            wout = wpool.tile([128, 32], dt.bfloat16, name="wout", tag="wout")
            fcint = wpool.tile([FB, H], dt.bfloat16, name="fcint", tag="fcint")
            h_all = hpool.tile([128, 4 * BL], dt.float32, name="hall", tag="hall")
            hb = hpool.tile([128, 4, BL], dt.bfloat16, name="hb", tag="hb")
            hb8 = hpool.tile([128, 4, BL], dt.float8e4, name="hb8", tag="hb8")

            CW = SPB * BL  # columns per block

            # early DMAs: block-0 inputs + projection weights first so PE can
            # start the block-0 projection while the big weight DMAs land.
            nc.sync.dma_start(stg[:FB, 0:CW], xdat_d[:, 0:CW])
            nc.sync.dma_start(fcint[:FB, :], fcin_d[:])
            # DVE funnel: PE Matmult supports only ONE embedded sem wait, so
            # route DMA-landed matmul operands through DVE; matmul deps then
            # collapse onto the single DVE semaphore.
            nc.vector.tensor_copy(fcin[:FB, :], fcint[:FB, :])
            nc.sync.dma_start(stg[:FB, CW:2 * CW], xdat_d[:, CW:2 * CW])
            for c in range(4):
                nc.sync.dma_start(h_all[:, c * BL:(c + 1) * BL],
                                  h0_d[c * 128:(c + 1) * 128, :])
            nc.vector.tensor_copy(hb[:], h_all[:])
            nc.vector.tensor_scalar_mul(hb8[:], h_all[:], SX)
            nc.sync.dma_start(brow[:], brow_d[:])
            nc.sync.dma_start(sel[:], sel_d[:])
            nc.sync.dma_start(bni[:], bni_d[:])
            nc.sync.dma_start(brown[:], brown_d[:])
            nc.sync.dma_start(wout[:], wout_d[:])
            nc.sync.dma_start(whh[:], whh_d[:])
            for p in range(2):
                nc.sync.dma_start(wi[p][:], wi_d[p][:])
                nc.sync.dma_start(wh8[p][:], wh8_d[p][:])
                nc.sync.dma_start(whn8[p][:], whn8_d[p][:])

            def whn(kc, c4):
                # Whh.T n-gate block: contraction chunk kc, out-chunk c4
                return whh[:, kc * H + c4 * 128: kc * H + (c4 + 1) * 128]

            def proj_col(off, xb_t, s):
                # two oc per PSUM bank, ONE accumulation group per bank,
                # then one full-bank relu evicting both chunks as scaled fp8
                for half in range(2):
                    pj = ppool.tile([128, 2 * BL], dt.float32, name="gpj",
                                    tag="gp", bufs=2)
                    for oc2 in range(2):
                        oc = half * 2 + oc2
                        nc.tensor.matmul(pj[:, oc2 * BL:(oc2 + 1) * BL],
                                         fcin[:FB, oc * 128:(oc + 1) * 128],
                                         stg[:FB, off + s * BL:off + (s + 1) * BL],
                                         start=(oc2 == 0), stop=(oc2 == 1),
                                         skip_group_check=True)
                    nc.scalar.activation(
                        xb_t[:, 2 * half:2 * half + 2, s * BL:(s + 1) * BL],
                        pj[:], AF.Relu, scale=SX)

            # block 0 projects its own inputs up front (PE filler during the
            # weight DMAs); later blocks are projected inside the prior block
            xb_cur = xpool.tile([128, 4, CW], dt.float8e4, name="xb8",
                                tag="xb8", bufs=2)
            for s in range(SPB):
                proj_col(0, xb_cur, s)

            pend = None
            for blk in range(NBLK):
                if blk + 1 < NBLK:
                    xb_next = xpool.tile([128, 4, CW], dt.float8e4, name="xb8",
                                         tag="xb8", bufs=2)
                if blk + 2 < NBLK:
                    nc.sync.dma_start(stg[:FB, (blk + 2) * CW:(blk + 3) * CW],
                                      xdat_d[:, (blk + 2) * CW:(blk + 3) * CW])
                ysb = tpool.tile([1, SPB * BL], dt.float32, name="ysb",
                                 tag="ysb", bufs=2)

                for s in range(SPB):
                    def xsl(p):
                        return xb_cur[:, 2 * p:2 * p + 2, s * BL:(s + 1) * BL]

                    # ---- x-phase: banks in the order chain(t-1) frees them
                    # r/z x-side + K=2 bias row opens each bank's group;
                    # rz banks were freed by the earliest chain events
                    # (the sigmoids), so PE restarts immediately.
                    rz_ps = []
                    for bk in range(4):  # banks: r01, r23, z01, z23
                        gp = ppool.tile([128, 2 * BL], dt.float32, name="grz",
                                        tag="grz", bufs=4)
                        nc.tensor.matmul(gp[:], brow[:, bk * 128:(bk + 1) * 128],
                                         sel[:], start=True, stop=False,
                                         skip_group_check=True)
                        for c2 in range(2):
                            m = bk * 2 + c2
                            ms = slice(m * 128, (m + 1) * 128)
                            for p in range(2):
                                nc.tensor.matmul(
                                    gp[:, c2 * BL:(c2 + 1) * BL],
                                    wi[p][:, :, ms], xsl(p),
                                    start=False, stop=False,
                                    perf_mode=DR, skip_group_check=True)
                        rz_ps.append(gp)

                    # next block's projection (gp banks, freed by rhn)
                    if blk + 1 < NBLK:
                        proj_col((blk + 1) * CW, xb_next, s)

                    # deferred output head for the previous step (gp bank)
                    if pend is not None:
                        pysb, pblk, ps_ = pend
                        yp = ppool.tile([128, 2 * BL], dt.float32, name="yp",
                                        tag="gp", bufs=2)
                        for c in range(4):
                            w0 = ps_ * 4 + c
                            nc.tensor.matmul(yp[0:1, 0:BL],
                                             wout[:, w0:w0 + 1],
                                             hb[:, c, :],
                                             start=(c == 0), stop=(c == 3),
                                             skip_group_check=True)
                        nc.scalar.activation(pysb[0:1, ps_ * BL:(ps_ + 1) * BL],
                                             yp[0:1, 0:BL], AF.Copy)
                        if ps_ == SPB - 1:
                            nc.sync.dma_start(
                                yout_d[pblk * SPB:(pblk + 1) * SPB, :],
                                pysb[0:1, :])
                        pend = None

                    # n-gate input side (gin): own banks, freed last (by sa)
                    gin_ps = []
                    for pr in range(2):
                        gp = ppool.tile([128, 2 * BL], dt.float32, name="gin",
                                        tag="gin", bufs=2)
                        for c2 in range(2):
                            c4 = pr * 2 + c2
                            ms = slice((8 + c4) * 128, (9 + c4) * 128)
                            for p in range(2):
                                nc.tensor.matmul(
                                    gp[:, c2 * BL:(c2 + 1) * BL],
                                    wi[p][:, :, ms], xsl(p),
                                    start=(c2 == 0 and p == 0),
                                    stop=(c2 == 1 and p == 1),
                                    perf_mode=DR, skip_group_check=True)
                        gin_ps.append(gp)

                    # ---- h-phase (bf16): r banks + ghn first so the chain
                    # starts early; z banks feed the late h-update
                    r_all = tpool.tile([128, 4 * BL], dt.float32, name="rall",
                                       tag="rall", bufs=2)
                    z_all = tpool.tile([128, 4 * BL], dt.float32, name="zall",
                                       tag="zall", bufs=2)
                    zp_all = tpool.tile([128, 4 * BL], dt.float32, name="zpall",
                                        tag="zpall", bufs=2)
                    ghn_ps = []

                    def rz_h(bk):
                        gp = rz_ps[bk]
                        for p in range(2):
                            for c2 in range(2):
                                m = bk * 2 + c2
                                ms = slice(m * 128, (m + 1) * 128)
                                nc.tensor.matmul(
                                    gp[:, c2 * BL:(c2 + 1) * BL],
                                    wh8[p][:, :, ms],
                                    hb8[:, 2 * p:2 * p + 2, :],
                                    start=False,
                                    stop=(c2 == 1 and p == 1),
                                    perf_mode=DR, skip_group_check=True)
                        o = (bk % 2) * 2 * BL
                        if bk < 2:
                            nc.scalar.activation(r_all[:, o:o + 2 * BL], gp[:],
                                                 AF.Sigmoid, scale=DS)
                        else:
                            nc.scalar.activation(z_all[:, o:o + 2 * BL], gp[:],
                                                 AF.Sigmoid, scale=DS)
                            nc.scalar.activation(zp_all[:, o:o + 2 * BL], gp[:],
                                                 AF.Sigmoid, scale=-DS)

                    def ghn_h(pr):
                        gp = ppool.tile([128, 2 * BL], dt.float32, name="ghn",
                                        tag="gp", bufs=2)
                        nc.tensor.matmul(gp[:], brown[:, pr * 128:(pr + 1) * 128],
                                         sel[:], start=True, stop=False,
                                         skip_group_check=True)
                        for c2 in range(2):
                            c4 = pr * 2 + c2
                            ms = slice(c4 * 128, (c4 + 1) * 128)
                            for p in range(2):
                                nc.tensor.matmul(
                                    gp[:, c2 * BL:(c2 + 1) * BL],
                                    whn8[p][:, :, ms],
                                    hb8[:, 2 * p:2 * p + 2, :],
                                    start=False,
                                    stop=(c2 == 1 and p == 1),
                                    perf_mode=DR, skip_group_check=True)
                        ghn_ps.append(gp)

                    rz_h(0)      # r01 -> sigmoid r01
                    ghn_h(0)     # ghn01
                    rz_h(1)      # r23 -> sigmoid r23
                    ghn_h(1)     # ghn23
                    rz_h(2)      # z01
                    rz_h(3)      # z23

                    t1 = tpool.tile([128, 4 * BL], dt.float32, name="t1",
                                    tag="t1", bufs=2)
                    for c4 in range(4):
                        cs = slice(c4 * BL, (c4 + 1) * BL)
                        nc.gpsimd.tensor_mul(t1[:, cs], z_all[:, cs],
                                             h_all[:, cs])

                    # ---- chain: rhn = ghn*r (full-bank, bnh already in
                    # psum via the bias row), sa = gin + rhn (full-bank),
                    # tanh per chunk (bias bni), then the ladder-free update
                    # h' = z*h + (1-z)*n with 1-z = sigmoid(-x) from the same
                    # bank; hb recast per pair so pair0 overlaps pair1
                    sa_all = tpool.tile([128, 4 * BL], dt.float32, name="saall",
                                        tag="saall", bufs=2)
                    nsb_all = tpool.tile([128, 4 * BL], dt.float32, name="nsball",
                                         tag="nsball", bufs=2)
                    t2 = tpool.tile([128, 4 * BL], dt.float32, name="t2",
                                    tag="t2", bufs=2)
                    # all four full-bank PSUM reads lead the DVE stream so
                    # pair1's chain starts as soon as its banks stop (not
                    # queued behind pair0's elementwise tail)
                    for pr in range(2):
                        o = pr * 2 * BL
                        rhn = tpool.tile([128, 2 * BL], dt.float32, name="rhn",
                                         tag="rhn", bufs=2)
                        nc.vector.tensor_mul(rhn[:], ghn_ps[pr][:],
                                             r_all[:, o:o + 2 * BL])
                        nc.vector.tensor_add(sa_all[:, o:o + 2 * BL],
                                             gin_ps[pr][:], rhn[:])
                    for pr in range(2):
                        for c2 in range(2):
                            c4 = pr * 2 + c2
                            cs = slice(c4 * BL, (c4 + 1) * BL)
                            nc.scalar.activation(nsb_all[:, cs], sa_all[:, cs],
                                                 AF.Tanh, bias=bni[:, c4:c4 + 1],
                                                 scale=DS)
                    for pr in range(2):
                        for c2 in range(2):
                            c4 = pr * 2 + c2
                            cs = slice(c4 * BL, (c4 + 1) * BL)
                            nc.vector.tensor_mul(t2[:, cs], zp_all[:, cs],
                                                 nsb_all[:, cs])
                            nc.vector.tensor_add(h_all[:, cs], t1[:, cs],
                                                 t2[:, cs])
                        # hb8 gates the next step's rz-h matmuls: cast first
                        nc.vector.tensor_scalar_mul(
                            hb8[:, 2 * pr:2 * pr + 2, :],
                            h_all[:, pr * 2 * BL:(pr + 1) * 2 * BL], SX)
                    # bf16 hb only feeds the head (next x-phase): one batched
                    # copy after the chain, off the cast8 critical path
                    nc.vector.tensor_copy(hb[:], h_all[:])
                    pend = (ysb, blk, s)

                if blk + 1 < NBLK:
                    xb_cur = xb_next

            # drain the final step's head
            pysb, pblk, ps_ = pend
            yp = ppool.tile([128, 2 * BL], dt.float32, name="yp", tag="gp",
                            bufs=2)
            for c in range(4):
                w0 = ps_ * 4 + c
                nc.tensor.matmul(yp[0:1, 0:BL], wout[:, w0:w0 + 1],
                                 hb[:, c, :], start=(c == 0), stop=(c == 3),
                                 skip_group_check=True)
            nc.scalar.activation(pysb[0:1, ps_ * BL:(ps_ + 1) * BL],
                                 yp[0:1, 0:BL], AF.Copy)
            nc.sync.dma_start(yout_d[pblk * SPB:(pblk + 1) * SPB, :],
                              pysb[0:1, :])

    _legalize(nc, mybir)
    return nc


def _get_nc():
    if "nc" not in _CACHE:
        _CACHE["nc"] = _build()
    return _CACHE["nc"]


def _wsp(w):
    chunks = w.reshape(4, 128)
    out = np.zeros((128, 32), np.float32)
    for s in range(SPB):
        for c in range(4):
            out[:, s * 4 + c] = chunks[c]
    return out.astype(BF16)


def _pack_dr(wT):
    """[512, 1536] contraction-major weight -> two DoubleRow pair tensors
    [128, 2*1536] fp8: pair p rows (2p,2p+1); [k, i*1536+m] = wT[p*256+i*128+k, m]."""
    w = (wT * SW).astype(np.float32)
    out = []
    for p in range(2):
        t = np.empty((128, 2 * 3 * H), np.float32)
        for i in range(2):
            t[:, i * 3 * H:(i + 1) * 3 * H] = w[(2 * p + i) * 128:
                                                (2 * p + i + 1) * 128, :]
        out.append(np.ascontiguousarray(t).astype(F8))
    return out


def kernel(data, init, fc_in_W, fc_in_b, Wih, Whh, bih, bhh, fc_out_W, fc_out_b):
    from concourse.bass_utils import run_bass_kernel_spmd

    data = np.asarray(data, np.float32)
    init = np.asarray(init, np.float32)
    fc_in_W = np.asarray(fc_in_W, np.float32)
    fc_in_b = np.asarray(fc_in_b, np.float32)
    Wih = np.asarray(Wih, np.float32)
    Whh = np.asarray(Whh, np.float32)
    bih = np.asarray(bih, np.float32)
    bhh = np.asarray(bhh, np.float32)
    fc_out_W = np.asarray(fc_out_W, np.float32)
    fc_out_b = np.asarray(fc_out_b, np.float32)

    nc = _get_nc()

    sel = np.zeros((2, 512), np.float32)
    sel[0, :256] = 1.0
    sel[1, 256:] = 1.0
    sel = sel.astype(BF16)

    in_maps = []
    for i in range(8):
        k, j = i // 2, i % 2
        d = data[j * BL:(j + 1) * BL]            # [256, 64, 64] (b,f,t)
        if k >= 2:
            d = d[:, :, ::-1]                    # reversed-time branches
        xdat = np.ascontiguousarray(d.transpose(1, 2, 0)).reshape(F, T * BL)
        xdat = np.concatenate([xdat, np.ones((1, T * BL), np.float32)], axis=0)
        fcin = np.concatenate([fc_in_W[k].T, fc_in_b[k][None, :]], axis=0)
        brz = (bih[k][:2 * H] + bhh[k][:2 * H]) * PS     # [1024]
        brow = brz.reshape(8, 128)               # chunk-major
        brow2 = np.empty((2, 512), np.float32)
        for bk in range(4):
            for i2 in range(2):
                brow2[i2, bk * 128:(bk + 1) * 128] = brow[2 * bk + i2]
        bnhr = (bhh[k][2 * H:] * PS).reshape(4, 128)
        brown_n = np.empty((2, 256), np.float32)
        for pr in range(2):
            for i2 in range(2):
                brown_n[i2, pr * 128:(pr + 1) * 128] = bnhr[2 * pr + i2]
        brown_n = np.ascontiguousarray(brown_n).astype(BF16)
        wip = _pack_dr(Wih[k].T)
        whT_rz = (Whh[k].T[:, :2 * H] * SW).astype(np.float32)  # [512, 1024]
        wh8p = []
        for p in range(2):
            t = np.empty((128, 2 * 2 * H), np.float32)
            for i2 in range(2):
                t[:, i2 * 2 * H:(i2 + 1) * 2 * H] = whT_rz[(2 * p + i2) * 128:
                                                           (2 * p + i2 + 1) * 128]
            wh8p.append(np.ascontiguousarray(t).astype(F8))
        whT_n = (Whh[k].T[:, 2 * H:] * SW).astype(np.float32)  # [512, 512]
        whn8p = []
        for p in range(2):
            t = np.empty((128, 2 * H), np.float32)
            for i2 in range(2):
                t[:, i2 * H:(i2 + 1) * H] = whT_n[(2 * p + i2) * 128:
                                                  (2 * p + i2 + 1) * 128]
            whn8p.append(np.ascontiguousarray(t).astype(F8))
        # whh: n cols only, [128, kc*512 + m] = Whh.T[kc*128+kk, 2H+m] * PS
        whT = (Whh[k].T[:, 2 * H:] * PS).astype(np.float32)  # [512, 512]
        whh = np.empty((128, 4 * H), np.float32)
        for kc in range(4):
            whh[:, kc * H:(kc + 1) * H] = whT[kc * 128:(kc + 1) * 128]
        in_maps.append({
            "xdat": np.ascontiguousarray(xdat).astype(BF16),
            "fcin": np.ascontiguousarray(fcin).astype(BF16),  # [65, 512]
            "wi0": wip[0], "wi1": wip[1],
            "wh80": wh8p[0], "wh81": wh8p[1],
            "whn80": whn8p[0], "whn81": whn8p[1],
            "whh": np.ascontiguousarray(whh).astype(BF16),
            "brow": np.ascontiguousarray(brow2).astype(BF16),
            "sel": sel,
            "bni": np.ascontiguousarray(bih[k][2 * H:].reshape(4, 128).T),
            "brown": brown_n,
            "h0": np.ascontiguousarray(init[j * BL:(j + 1) * BL].T),
            "wout": _wsp(fc_out_W[k % 2]),
        })

    kw = {"trace": True} if TRACE else {}
    res = run_bass_kernel_spmd(nc, in_maps, list(range(8)), **kw)
    LAST["res"] = res
    y = [np.asarray(res.results[i]["yout"], np.float32) for i in range(8)]

    air_out = np.empty((B, T), np.float32)
    bed_out = np.empty((B, T), np.float32)
    for j in range(2):
        sl = slice(j * BL, (j + 1) * BL)
        air_out[sl] = (y[0 + j] + y[4 + j][::-1]).T + fc_out_b[0]
        bed_out[sl] = (y[2 + j] + y[6 + j][::-1]).T + fc_out_b[1]
    return air_out, bed_out
